# revision 1
# baseline (speedup 1.0000x reference)
"""Trainium2 Bass kernel for DecomposableAttention (B=512, L=256, V=50000, E=300, H=200).

Strategy: data-parallel over batch across 8 cores (64 batches/core).  Per batch:
indirect-DMA gather of embedding rows, on-chip PE transposes to get the
E-on-partitions layout, f32r matmuls for the attend/compare MLPs and the
attention einsums, ACT-exp softmaxes with the length masks folded in as
per-partition -30000 biases, and a final aggregate MLP over all 64 batches.
All matmul free dims are 256 so f32r runs at 1 cycle/row.
"""
import sys

if '/opt/trn_rl_repo' not in sys.path:
    sys.path.insert(0, '/opt/trn_rl_repo')

import numpy as np

B, L, VOCAB, EMBED, HIDDEN = 512, 256, 50000, 300, 200
NCORES = 8
BC = B // NCORES  # batches per core

_prog_cache = {}
USE_F32R = False


def build_program(nb=BC):
    import concourse.bass as bass
    import concourse.bacc as bacc
    import concourse.tile as tile
    import concourse.mybir as mybir
    from concourse.masks import make_identity

    F32 = mybir.dt.float32
    F32R = mybir.dt.float32r if USE_F32R else mybir.dt.float32
    I32 = mybir.dt.int32
    AX = mybir.AxisListType
    ALU = mybir.AluOpType
    ACTF = mybir.ActivationFunctionType
    P = 128
    EK = [(0, 128), (128, 256), (256, 300)]      # E contraction chunks
    H2 = [(0, 100), (100, 200)]                  # H chunks of 100
    E3 = [(0, 100), (100, 200), (200, 300)]      # E output chunks of 100

    nc = bacc.Bacc("TRN2", num_devices=NCORES)

    emb_d = nc.dram_tensor("emb", [VOCAB, EMBED], F32, kind="ExternalInput")
    s1_d = nc.dram_tensor("s1", [nb, L], I32, kind="ExternalInput")
    s2_d = nc.dram_tensor("s2", [nb, L], I32, kind="ExternalInput")
    len1_d = nc.dram_tensor("len1", [nb, 1], I32, kind="ExternalInput")
    len2_d = nc.dram_tensor("len2", [nb, 1], I32, kind="ExternalInput")
    W1a_d = nc.dram_tensor("W1a", [EMBED, HIDDEN], F32R, kind="ExternalInput")
    W2a_d = nc.dram_tensor("W2a", [HIDDEN, HIDDEN], F32R, kind="ExternalInput")
    W1c_d = nc.dram_tensor("W1c", [2 * EMBED, HIDDEN], F32R, kind="ExternalInput")
    W2c_d = nc.dram_tensor("W2c", [HIDDEN, HIDDEN], F32R, kind="ExternalInput")
    W1g_d = nc.dram_tensor("W1g", [2 * HIDDEN, HIDDEN], F32R, kind="ExternalInput")
    W2g_d = nc.dram_tensor("W2g", [HIDDEN, 2], F32R, kind="ExternalInput")
    b1a_d = nc.dram_tensor("b1a", [HIDDEN, 1], F32, kind="ExternalInput")
    b2a_d = nc.dram_tensor("b2a", [HIDDEN, 1], F32, kind="ExternalInput")
    b1c_d = nc.dram_tensor("b1c", [HIDDEN, 1], F32, kind="ExternalInput")
    b2c_d = nc.dram_tensor("b2c", [HIDDEN, 1], F32, kind="ExternalInput")
    b1g_d = nc.dram_tensor("b1g", [HIDDEN, 1], F32, kind="ExternalInput")
    b2g_d = nc.dram_tensor("b2g", [2, 1], F32, kind="ExternalInput")
    out_d = nc.dram_tensor("out", [nb, 2], F32, kind="ExternalOutput")

    with tile.TileContext(nc) as tc:
        import contextlib
        ctx = contextlib.ExitStack()
        with ctx:
            const = ctx.enter_context(tc.tile_pool(name="const", bufs=1))
            psA = ctx.enter_context(tc.tile_pool(name="psA", bufs=3, space="PSUM"))
            psS = ctx.enter_context(tc.tile_pool(name="psS", bufs=3, space="PSUM"))
            psT = ctx.enter_context(tc.tile_pool(name="psT", bufs=2, space="PSUM"))
            gat = ctx.enter_context(tc.tile_pool(name="gat", bufs=3))
            eTp = ctx.enter_context(tc.tile_pool(name="eTp", bufs=2))
            hp = ctx.enter_context(tc.tile_pool(name="hp", bufs=2))
            sm = ctx.enter_context(tc.tile_pool(name="sm", bufs=2))
            att = ctx.enter_context(tc.tile_pool(name="att", bufs=2))
            cmp_ = ctx.enter_context(tc.tile_pool(name="cmp", bufs=2))

            # ---------------- constants ----------------
            ident = const.tile([P, P], F32)
            make_identity(nc, ident[:])
            ident_r = const.tile([P, P], F32R)
            nc.vector.tensor_copy(ident_r[:], ident[:])

            ones_f = const.tile([P, 1], F32)
            nc.vector.memset(ones_f[:], 1.0)
            ones_col_r = const.tile([P, 1], F32R)   # lhsT for den sums (K=128, M=1)
            nc.vector.tensor_copy(ones_col_r[:], ones_f[:])
            ones_row_f = const.tile([1, P], F32)
            nc.vector.memset(ones_row_f[:], 1.0)
            ones_row_r = const.tile([1, P], F32R)   # lhsT for bcasts (K=1, M=128)
            nc.vector.tensor_copy(ones_row_r[:], ones_row_f[:])

            # weights
            W1a_t = [const.tile([k1 - k0, HIDDEN], F32R, name=f"W1a{i}", tag=f"W1a{i}") for i, (k0, k1) in enumerate(EK)]
            for i, (k0, k1) in enumerate(EK):
                nc.sync.dma_start(W1a_t[i][:], W1a_d[k0:k1, :])
            W2a_t = [const.tile([100, HIDDEN], F32R, name=f"W2a{i}", tag=f"W2a{i}") for i in range(2)]
            for i, (k0, k1) in enumerate(H2):
                nc.sync.dma_start(W2a_t[i][:], W2a_d[k0:k1, :])
            W1ca_t = [const.tile([k1 - k0, HIDDEN], F32R, name=f"W1ca{i}", tag=f"W1ca{i}") for i, (k0, k1) in enumerate(EK)]
            for i, (k0, k1) in enumerate(EK):
                nc.sync.dma_start(W1ca_t[i][:], W1c_d[k0:k1, :])
            W1cb_t = [const.tile([100, HIDDEN], F32R, name=f"W1cb{i}", tag=f"W1cb{i}") for i in range(3)]
            for i, (k0, k1) in enumerate(E3):
                nc.sync.dma_start(W1cb_t[i][:], W1c_d[EMBED + k0:EMBED + k1, :])
            W2c_t = [const.tile([100, HIDDEN], F32R, name=f"W2c{i}", tag=f"W2c{i}") for i in range(2)]
            for i, (k0, k1) in enumerate(H2):
                nc.sync.dma_start(W2c_t[i][:], W2c_d[k0:k1, :])
            W1g_t = [const.tile([100, HIDDEN], F32R, name=f"W1g{i}", tag=f"W1g{i}") for i in range(4)]
            for i in range(4):
                nc.sync.dma_start(W1g_t[i][:], W1g_d[i * 100:(i + 1) * 100, :])
            W2g_t = [const.tile([100, 2], F32R, name=f"W2g{i}", tag=f"W2g{i}") for i in range(2)]
            for i, (k0, k1) in enumerate(H2):
                nc.sync.dma_start(W2g_t[i][:], W2g_d[k0:k1, :])

            def bias2(d):
                t = [const.tile([100, 1], F32, name=f"b{d.name}{i}", tag=f"b{d.name}{i}") for i in range(2)]
                for i, (k0, k1) in enumerate(H2):
                    nc.sync.dma_start(t[i][:], d[k0:k1, :])
                return t
            b1a_t, b2a_t = bias2(b1a_d), bias2(b2a_d)
            b1c_t, b2c_t = bias2(b1c_d), bias2(b2c_d)
            b1g_t = bias2(b1g_d)
            b2g_t = const.tile([2, 1], F32)
            nc.sync.dma_start(b2g_t[:], b2g_d[:])

            # masks / lengths
            len1_t = const.tile([nb, 1], I32)
            nc.sync.dma_start(len1_t[:], len1_d[:])
            len2_t = const.tile([nb, 1], I32)
            nc.sync.dma_start(len2_t[:], len2_d[:])
            iota_t = const.tile([nb, L], I32)
            nc.gpsimd.iota(iota_t[:], pattern=[[1, L]], base=0, channel_multiplier=0)

            mask_all = []     # f32 [nb, L] per sentence
            lmT = []          # transposed logmasks: per sentence, 2 tiles [128, nb] f32
            for s, lent in ((0, len1_t), (1, len2_t)):
                m = const.tile([nb, L], F32, name=f"mask{s}", tag=f"mask{s}")
                nc.vector.tensor_tensor(m[:], iota_t[:], lent[:].to_broadcast([nb, L]), op=ALU.is_lt)
                lm = const.tile([nb, L], F32, name=f"lm{s}", tag=f"lm{s}")
                nc.vector.tensor_scalar(lm[:], m[:], 1.0, 30000.0, op0=ALU.subtract, op1=ALU.mult)
                lts = []
                for c in range(2):
                    tp = psT.tile([P, nb], F32, name="lmT_ps", tag="tr")
                    nc.tensor.transpose(tp[:], lm[:, c * P:(c + 1) * P], ident[:nb, :nb])
                    lt = const.tile([P, nb], F32, name=f"lmT{s}{c}", tag=f"lmT{s}{c}")
                    nc.vector.tensor_copy(lt[:], tp[:])
                    lts.append(lt)
                mask_all.append(m)
                lmT.append(lts)

            # per-batch masks are built as tiny [1, L] rows from iota_row + len_f
            len_f = []
            for s, ld in ((0, len1_d), (1, len2_d)):
                lf = const.tile([1, nb], I32, name=f"lenf{s}", tag=f"lenf{s}")
                nc.sync.dma_start(lf[:], ld[:].rearrange("n one -> one n"))
                len_f.append(lf)
            iota_row = const.tile([1, L], I32)
            nc.gpsimd.iota(iota_row[:], pattern=[[1, L]], base=0, channel_multiplier=0)

            # token indices, transposed to [128, nb] int32 per chunk
            sT = []
            for s, sd in ((0, s1_d), (1, s2_d)):
                st = const.tile([nb, L], I32, name=f"s{s}", tag=f"s{s}")
                nc.sync.dma_start(st[:], sd[:])
                sf = const.tile([nb, L], F32, name=f"sf{s}", tag=f"sf{s}")
                nc.vector.tensor_copy(sf[:], st[:])
                chunks = []
                for c in range(2):
                    tp = psT.tile([P, nb], F32, name="sT_ps", tag="tr")
                    nc.tensor.transpose(tp[:], sf[:, c * P:(c + 1) * P], ident[:nb, :nb])
                    tf = const.tile([P, nb], F32, name=f"sTf{s}{c}", tag=f"sTf{s}{c}")
                    nc.vector.tensor_copy(tf[:], tp[:])
                    ti = const.tile([P, nb], I32, name=f"sTi{s}{c}", tag=f"sTi{s}{c}")
                    nc.vector.tensor_copy(ti[:], tf[:])
                    chunks.append(ti)
                sT.append(chunks)

            # v accumulators [100, nb] per H-chunk per sentence
            v_all = [[const.tile([100, nb], F32, name=f"v{s}{m}", tag=f"v{s}{m}") for m in range(2)] for s in range(2)]

            # ---------------- per-batch loop ----------------
            for b in range(nb):
                # mask rows [1, L] for this batch (partition 0)
                mrow = []    # f32
                mrow_r = []  # f32r
                for s in range(2):
                    mr = sm.tile([1, L], F32, name=f"mrow{s}", tag=f"mrow{s}")
                    nc.vector.tensor_tensor(mr[:], iota_row[:],
                                            len_f[s][:, b:b + 1].to_broadcast([1, L]), op=ALU.is_lt)
                    mrr = sm.tile([1, L], F32R, name=f"mrowr{s}", tag=f"mrowr{s}")
                    nc.vector.tensor_copy(mrr[:], mr[:])
                    mrow.append(mr)
                    mrow_r.append(mrr)
                eR = [[], []]   # f32r natural [128, 300] x2 chunks per sentence
                eT = [[], []]   # f32r transposed [(128|128|44), 256] x3 per sentence
                hT = [[], []]   # f32r [100, 256] x2 per sentence
                for s in range(2):
                    for c in range(2):
                        en = gat.tile([P, EMBED], F32, name=f"eN{s}{c}", tag=f"eN{s}{c}")
                        nc.gpsimd.indirect_dma_start(
                            out=en[:], out_offset=None, in_=emb_d[:],
                            in_offset=bass.IndirectOffsetOnAxis(ap=sT[s][c][:, b:b + 1], axis=0),
                        )
                        if USE_F32R:
                            er = gat.tile([P, EMBED], F32R, name=f"eR{s}{c}", tag=f"eR{s}{c}")
                            nc.vector.tensor_copy(er[:], en[:])
                        else:
                            er = en
                        eR[s].append(er)
                    for k, (k0, k1) in enumerate(EK):
                        et = eTp.tile([k1 - k0, L], F32R, name=f"eT{s}{k}", tag=f"eT{s}{k}")
                        for c in range(2):
                            tp = psT.tile([P, P], F32R, name="tr_ps", tag="tr")
                            nc.tensor.transpose(tp[:k1 - k0, :], eR[s][c][:, k0:k1], ident_r[:])
                            nc.any.tensor_copy(et[:, c * P:(c + 1) * P], tp[:k1 - k0, :])
                        eT[s].append(et)
                # attend MLP for both sentences, interleaved so each weight
                # chunk's LDWEIGHTS is reused by the second sentence's matmul
                ha = [[], []]
                for m, (m0, m1) in enumerate(H2):
                    pp = [psA.tile([100, L], F32, name=f"h1_ps{s}", tag="mm") for s in range(2)]
                    for k in range(3):
                        for s in range(2):
                            nc.tensor.matmul(pp[s][:], W1a_t[k][:, m0:m1], eT[s][k][:],
                                             start=(k == 0), stop=(k == 2))
                    for s in range(2):
                        h = hp.tile([100, L], F32R, name=f"ha{s}{m}", tag=f"ha{s}{m}")
                        nc.scalar.activation(h[:], pp[s][:], ACTF.Relu, bias=b1a_t[m][:], scale=1.0)
                        ha[s].append(h)
                for m, (m0, m1) in enumerate(H2):
                    qp = [psA.tile([100, L], F32, name=f"h2_ps{s}", tag="mm") for s in range(2)]
                    for k2 in range(2):
                        for s in range(2):
                            nc.tensor.matmul(qp[s][:], W2a_t[k2][:, m0:m1], ha[s][k2][:],
                                             start=(k2 == 0), stop=(k2 == 1))
                    for s in range(2):
                        h = hp.tile([100, L], F32R, name=f"hT{s}{m}", tag=f"hT{s}{m}")
                        nc.scalar.activation(h[:], qp[s][:], ACTF.Relu, bias=b2a_t[m][:], scale=1.0)
                        hT[s].append(h)

                # scores: e [i, j] and e^T [j, i]; copy out of PSUM immediately
                # (frees score banks for the next batch's matmuls)
                e_sb, eT_sb = [], []
                for ic in range(2):
                    ep = psS.tile([P, L], F32, name=f"e_ps{ic}", tag="score")
                    for m in range(2):
                        nc.tensor.matmul(ep[:], hT[0][m][:, ic * P:(ic + 1) * P], hT[1][m][:],
                                         start=(m == 0), stop=(m == 1))
                    es = sm.tile([P, L], F32, name=f"e_sb{ic}", tag=f"e_sb{ic}")
                    nc.scalar.activation(es[:], ep[:], ACTF.Identity, bias=0.0, scale=1.0)
                    e_sb.append(es)
                for jc in range(2):
                    ep = psS.tile([P, L], F32, name=f"eT_ps{jc}", tag="score")
                    for m in range(2):
                        nc.tensor.matmul(ep[:], hT[1][m][:, jc * P:(jc + 1) * P], hT[0][m][:],
                                         start=(m == 0), stop=(m == 1))
                    es = sm.tile([P, L], F32, name=f"eT_sb{jc}", tag=f"eT_sb{jc}")
                    nc.scalar.activation(es[:], ep[:], ACTF.Identity, bias=0.0, scale=1.0)
                    eT_sb.append(es)

                # M = rowmax(e); broadcast tile Mb [128, 256]
                Mt_ps = psA.tile([1, L], F32, name="Mt_ps", tag="mm")
                for ic in range(2):
                    mp = sm.tile([P, 1], F32, name=f"M_p{ic}", tag=f"M_p{ic}")
                    nc.vector.tensor_reduce(mp[:], e_sb[ic][:], axis=AX.X, op=ALU.max)
                    nc.tensor.transpose(Mt_ps[:, ic * P:(ic + 1) * P], mp[:], ident[:])
                M_r = sm.tile([1, L], F32R, name="M_r", tag="M_r")
                nc.vector.tensor_copy(M_r[:], Mt_ps[:])
                Mb_ps = psA.tile([P, L], F32, name="Mb_ps", tag="mm")
                nc.tensor.matmul(Mb_ps[:], ones_row_r[:], M_r[:], start=True, stop=True)
                Mb = sm.tile([P, L], F32, name="Mb", tag="Mb")
                nc.any.tensor_copy(Mb[:], Mb_ps[:])

                # exp(e - M[j]) * mask1[i];  exp(eT - M[i]) * mask2[j]
                u = [[], []]  # u[0]=uA (i-part), u[1]=uB (j-part)
                for d, (eps, lmTs) in enumerate(((e_sb, lmT[0]), (eT_sb, lmT[1]))):
                    for c in range(2):
                        t = sm.tile([P, L], F32, name=f"t{d}{c}", tag=f"t{d}{c}")
                        nc.vector.tensor_tensor(t[:], eps[c][:], Mb[:], op=ALU.subtract)
                        uu = sm.tile([P, L], F32R, name=f"u{d}{c}", tag=f"u{d}{c}")
                        nc.scalar.activation(uu[:], t[:], ACTF.Exp, bias=lmTs[c][:, b:b + 1], scale=1.0)
                        u[d].append(uu)

                # denominators and normalizer broadcast tiles
                R_bc = []
                for d in range(2):
                    dp = psA.tile([1, L], F32, name=f"den_ps{d}", tag="mm")
                    for c in range(2):
                        nc.tensor.matmul(dp[:], ones_col_r[:], u[d][c][:],
                                         start=(c == 0), stop=(c == 1))
                    rc = sm.tile([1, L], F32, name=f"recip{d}", tag=f"recip{d}")
                    nc.vector.reciprocal(rc[:], dp[:])
                    rm = sm.tile([1, L], F32R, name=f"recipm{d}", tag=f"recipm{d}")
                    # direction A (d=0) masks output cols by mask2; B by mask1
                    nc.vector.tensor_tensor(rm[:], rc[:], mrow[1 - d][:], op=ALU.mult)
                    rp = psA.tile([P, L], F32, name=f"R_ps{d}", tag="mm")
                    nc.tensor.matmul(rp[:], ones_row_r[:], rm[:], start=True, stop=True)
                    rb = sm.tile([P, L], F32, name=f"R_bc{d}", tag=f"R_bc{d}")
                    nc.any.tensor_copy(rb[:], rp[:])
                    R_bc.append(rb)

                # attention sums: alphas^T = e1^T-chunks @ uA, betas^T = e2-chunks @ uB
                xT = [[], []]   # xT[0]=alphasT (for sentence-2 compare), xT[1]=betasT
                for d in range(2):
                    for m3, (m0, m1) in enumerate(E3):
                        ap_ = psA.tile([100, L], F32, name="attn_ps", tag="mm")
                        for c in range(2):
                            nc.tensor.matmul(ap_[:], eR[d][c][:, m0:m1], u[d][c][:],
                                             start=(c == 0), stop=(c == 1))
                        at = att.tile([100, L], F32R, name=f"xT{d}{m3}", tag=f"xT{d}{m3}")
                        nc.vector.tensor_tensor(at[:], ap_[:], R_bc[d][:100, :], op=ALU.mult)
                        xT[d].append(at)

                # compare MLP + masked sum.  sentence 1 pairs with betasT (xT[1]), mask1;
                # sentence 2 pairs with alphasT (xT[0]), mask2.
                # compare MLP, sentences interleaved per weight chunk.
                # sentence 0 pairs eT[0] with betasT (xT[1]); sentence 1 pairs
                # eT[1] with alphasT (xT[0]).
                r1 = [[], []]
                for m, (m0, m1) in enumerate(H2):
                    up = [psA.tile([100, L], F32, name=f"c1_ps{s}", tag="mm") for s in range(2)]
                    for k in range(3):
                        for s in range(2):
                            nc.tensor.matmul(up[s][:], W1ca_t[k][:, m0:m1], eT[s][k][:],
                                             start=(k == 0), stop=False)
                    for k3 in range(3):
                        for s in range(2):
                            nc.tensor.matmul(up[s][:], W1cb_t[k3][:, m0:m1], xT[1 - s][k3][:],
                                             start=False, stop=(k3 == 2))
                    for s in range(2):
                        r = cmp_.tile([100, L], F32R, name=f"r1{s}{m}", tag=f"r1{s}{m}")
                        nc.scalar.activation(r[:], up[s][:], ACTF.Relu, bias=b1c_t[m][:], scale=1.0)
                        r1[s].append(r)
                maskbc = []
                for s in range(2):
                    mb_ps = psA.tile([P, L], F32, name=f"maskbc_ps{s}", tag="mm")
                    nc.tensor.matmul(mb_ps[:], ones_row_r[:], mrow_r[s][:],
                                     start=True, stop=True)
                    mb_sb = cmp_.tile([100, L], F32, name=f"maskbc{s}", tag=f"maskbc{s}")
                    nc.any.tensor_copy(mb_sb[:], mb_ps[:100, :])
                    maskbc.append(mb_sb)
                for m, (m0, m1) in enumerate(H2):
                    cp = [psA.tile([100, L], F32, name=f"c2_ps{s}", tag="mm") for s in range(2)]
                    for k2 in range(2):
                        for s in range(2):
                            nc.tensor.matmul(cp[s][:], W2c_t[k2][:, m0:m1], r1[s][k2][:],
                                             start=(k2 == 0), stop=(k2 == 1))
                    for s in range(2):
                        c2 = cmp_.tile([100, L], F32, name=f"c2{s}{m}", tag=f"c2{s}{m}")
                        nc.scalar.activation(c2[:], cp[s][:], ACTF.Relu, bias=b2c_t[m][:], scale=1.0)
                        scr = cmp_.tile([100, L], F32, name=f"scr{s}{m}", tag=f"scr{s}{m}")
                        nc.vector.tensor_tensor(scr[:], c2[:], maskbc[s][:], op=ALU.mult)
                        nc.vector.tensor_reduce(v_all[s][m][:, b:b + 1], scr[:], axis=AX.X, op=ALU.add)

            # ---------------- aggregate ----------------
            vr = []
            for s in range(2):
                for m in range(2):
                    t = const.tile([100, nb], F32R, name=f"vr{s}{m}", tag=f"vr{s}{m}")
                    nc.vector.tensor_copy(t[:], v_all[s][m][:])
                    vr.append(t)
            g1 = []
            for m, (m0, m1) in enumerate(H2):
                gp = psA.tile([100, nb], F32, name="g_ps", tag="mm")
                for k in range(4):
                    nc.tensor.matmul(gp[:], W1g_t[k][:, m0:m1], vr[k][:],
                                     start=(k == 0), stop=(k == 3))
                g = const.tile([100, nb], F32R, name=f"g1{m}", tag=f"g1{m}")
                nc.scalar.activation(g[:], gp[:], ACTF.Relu, bias=b1g_t[m][:], scale=1.0)
                g1.append(g)
            op = psA.tile([2, nb], F32, name="o_ps", tag="mm")
            for k2 in range(2):
                nc.tensor.matmul(op[:], W2g_t[k2][:], g1[k2][:],
                                 start=(k2 == 0), stop=(k2 == 1))
            osb = const.tile([2, nb], F32, name="osb", tag="osb")
            nc.scalar.activation(osb[:], op[:], ACTF.Identity, bias=b2g_t[:], scale=1.0)
            nc.sync.dma_start(out_d[:].rearrange("b o -> o b"), osb[:])

    nc.compile()
    return nc


def _shard_inputs(inputs, nb=BC, ncores=NCORES):
    f = np.ascontiguousarray
    maps = []
    for c in range(ncores):
        sl = slice(c * nb, (c + 1) * nb)
        maps.append(dict(
            emb=f(inputs['emb'].astype(np.float32)),
            s1=f(inputs['s1'][sl].astype(np.int32)),
            s2=f(inputs['s2'][sl].astype(np.int32)),
            len1=f(inputs['len1'][sl].reshape(nb, 1).astype(np.int32)),
            len2=f(inputs['len2'][sl].reshape(nb, 1).astype(np.int32)),
            W1a=f(inputs['W1a'].astype(np.float32)),
            W2a=f(inputs['W2a'].astype(np.float32)),
            W1c=f(inputs['W1c'].astype(np.float32)),
            W2c=f(inputs['W2c'].astype(np.float32)),
            W1g=f(inputs['W1g'].astype(np.float32)),
            W2g=f(inputs['W2g'].astype(np.float32)),
            b1a=f(inputs['b1a'].reshape(-1, 1).astype(np.float32)),
            b2a=f(inputs['b2a'].reshape(-1, 1).astype(np.float32)),
            b1c=f(inputs['b1c'].reshape(-1, 1).astype(np.float32)),
            b2c=f(inputs['b2c'].reshape(-1, 1).astype(np.float32)),
            b1g=f(inputs['b1g'].reshape(-1, 1).astype(np.float32)),
            b2g=f(inputs['b2g'].reshape(-1, 1).astype(np.float32)),
        ))
    return maps


def kernel(**inputs):
    from concourse.bass_utils import run_bass_kernel_spmd
    if 'prog' not in _prog_cache:
        _prog_cache['prog'] = build_program(BC)
    nc = _prog_cache['prog']
    in_maps = _shard_inputs(inputs)
    res = run_bass_kernel_spmd(nc, in_maps, core_ids=list(range(NCORES)))
    out = np.concatenate([res.results[c]["out"] for c in range(NCORES)], axis=0)
    return out.astype(np.float32)



# revision 7
# speedup vs baseline: 2.1627x; 2.1627x over previous
"""Trainium2 Bass kernel for DecomposableAttention (B=512, L=256, V=50000, E=300, H=200).

Data-parallel over batch across 8 cores (64 batches/core).  All matmuls run in
bf16 (1 cycle/row on the PE vs 4 for fp32) with fp32 PSUM accumulation.
Batches are processed in groups of 4 so the attend/compare MLPs run at
N=512 free dim with the stationary weights reused across consecutive matmuls.

Softmax uses a per-batch *global* max offset (exact softmax invariance per
the reference's own offset freedom), so the max never needs a per-column
broadcast: exp reads score PSUM directly with the length mask and -G folded
into the per-partition ACT bias.  Masked positions gather a zero embedding
row (host-side index masking onto an appended zero row), which zeroes the
compare-MLP inputs at masked columns; the resulting constant output column
is subtracted at the end via a (256-len)*c0 rank-1 correction.
"""
import sys

if '/opt/trn_rl_repo' not in sys.path:
    sys.path.insert(0, '/opt/trn_rl_repo')

import numpy as np

B, L, VOCAB, EMBED, HIDDEN = 512, 256, 50000, 300, 200
NCORES = 8
NB = B // NCORES          # batches per core
GB = 4                    # batches per group
NG = NB // GB             # groups
VZERO = VOCAB             # index of the appended all-zero embedding row

_prog_cache = {}


def build_program():
    import concourse.bass as bass
    import concourse.bass_isa as bass_isa
    import concourse.bacc as bacc
    import concourse.tile as tile
    import concourse.mybir as mybir
    from concourse.masks import make_identity

    F32 = mybir.dt.float32
    BF16 = mybir.dt.bfloat16
    I32 = mybir.dt.int32
    AX = mybir.AxisListType
    ALU = mybir.AluOpType
    ACTF = mybir.ActivationFunctionType
    P = 128
    EK = [(0, 128), (128, 256), (256, 300)]     # E contraction chunks
    H2 = [(0, 100), (100, 200)]                 # H chunks of 100

    nc = bacc.Bacc("TRN2", num_devices=NCORES)

    emb_d = nc.dram_tensor("emb", [VOCAB + 1, EMBED], BF16, kind="ExternalInput")
    sT_d = nc.dram_tensor("sT", [P, 4 * NB], I32, kind="ExternalInput")
    lmT_d = nc.dram_tensor("lmT", [P, 4 * NB], F32, kind="ExternalInput")
    lenf_d = nc.dram_tensor("lenf", [1, 2 * NB], I32, kind="ExternalInput")
    lmg_d = nc.dram_tensor("lmg", [1, 2 * NB], BF16, kind="ExternalInput")
    W1a_d = nc.dram_tensor("W1a", [EMBED, HIDDEN], BF16, kind="ExternalInput")
    W2a_d = nc.dram_tensor("W2a", [HIDDEN, HIDDEN], BF16, kind="ExternalInput")
    W1c_d = nc.dram_tensor("W1c", [2 * EMBED, HIDDEN], BF16, kind="ExternalInput")
    W2c_d = nc.dram_tensor("W2c", [HIDDEN, HIDDEN], BF16, kind="ExternalInput")
    W1g_d = nc.dram_tensor("W1g", [2 * HIDDEN, HIDDEN], BF16, kind="ExternalInput")
    W2g_d = nc.dram_tensor("W2g", [HIDDEN, 2], BF16, kind="ExternalInput")
    b1a_d = nc.dram_tensor("b1a", [100, 2], F32, kind="ExternalInput")
    b2a_d = nc.dram_tensor("b2a", [100, 2], F32, kind="ExternalInput")
    b1c_d = nc.dram_tensor("b1c", [100, 2], F32, kind="ExternalInput")
    b2c_d = nc.dram_tensor("b2c", [100, 2], F32, kind="ExternalInput")
    b1g_d = nc.dram_tensor("b1g", [100, 2], F32, kind="ExternalInput")
    b2g_d = nc.dram_tensor("b2g", [2, 1], F32, kind="ExternalInput")
    out_d = nc.dram_tensor("out", [NB, 2], F32, kind="ExternalOutput")

    with tile.TileContext(nc) as tc:
        import contextlib
        ctx = contextlib.ExitStack()
        with ctx:
            const = ctx.enter_context(tc.tile_pool(name="const", bufs=1))
            gat = ctx.enter_context(tc.tile_pool(name="gat", bufs=2))
            eTp = ctx.enter_context(tc.tile_pool(name="eTp", bufs=2))
            hp = ctx.enter_context(tc.tile_pool(name="hp", bufs=2))
            up = ctx.enter_context(tc.tile_pool(name="up", bufs=2))
            sm = ctx.enter_context(tc.tile_pool(name="sm", bufs=2))
            cmp_ = ctx.enter_context(tc.tile_pool(name="cmp", bufs=2))
            # PSUM: two rings, 4 banks each (pools allocate bufs per TAG, so
            # every tile in a pool shares one tag).
            psMLP = ctx.enter_context(tc.tile_pool(name="psMLP", bufs=4, space="PSUM"))
            psSC = ctx.enter_context(tc.tile_pool(name="psSC", bufs=4, space="PSUM"))

            # ---------------- constants ----------------
            ident_f = const.tile([P, P], F32)
            make_identity(nc, ident_f[:])
            ident = const.tile([P, P], BF16)
            nc.vector.tensor_copy(ident[:], ident_f[:])

            ones_col = const.tile([P, 1], BF16)
            nc.vector.memset(ones_col[:], 1.0)

            iota_row = const.tile([1, L], I32)
            nc.gpsimd.iota(iota_row[:], pattern=[[1, L]], base=0, channel_multiplier=0)

            # weights (stationary tiles, bf16)
            W1a_t = [const.tile([k1 - k0, HIDDEN], BF16, name=f"W1a{i}", tag=f"W1a{i}")
                     for i, (k0, k1) in enumerate(EK)]
            for i, (k0, k1) in enumerate(EK):
                nc.sync.dma_start(W1a_t[i][:], W1a_d[k0:k1, :])
            W2a_t = [const.tile([100, HIDDEN], BF16, name=f"W2a{i}", tag=f"W2a{i}") for i in range(2)]
            for i, (k0, k1) in enumerate(H2):
                nc.sync.dma_start(W2a_t[i][:], W2a_d[k0:k1, :])
            W1ca_t = [const.tile([k1 - k0, HIDDEN], BF16, name=f"W1ca{i}", tag=f"W1ca{i}")
                      for i, (k0, k1) in enumerate(EK)]
            for i, (k0, k1) in enumerate(EK):
                nc.sync.dma_start(W1ca_t[i][:], W1c_d[k0:k1, :])
            W1cb_t = [const.tile([k1 - k0, HIDDEN], BF16, name=f"W1cb{i}", tag=f"W1cb{i}")
                      for i, (k0, k1) in enumerate(EK)]
            for i, (k0, k1) in enumerate(EK):
                nc.sync.dma_start(W1cb_t[i][:], W1c_d[EMBED + k0:EMBED + k1, :])
            W2c_t = [const.tile([100, HIDDEN], BF16, name=f"W2c{i}", tag=f"W2c{i}") for i in range(2)]
            for i, (k0, k1) in enumerate(H2):
                nc.sync.dma_start(W2c_t[i][:], W2c_d[k0:k1, :])
            W1g_t = [const.tile([100, HIDDEN], BF16, name=f"W1g{i}", tag=f"W1g{i}") for i in range(4)]
            for i in range(4):
                nc.sync.dma_start(W1g_t[i][:], W1g_d[i * 100:(i + 1) * 100, :])
            W2g_t = [const.tile([100, 2], BF16, name=f"W2g{i}", tag=f"W2g{i}") for i in range(2)]
            for i, (k0, k1) in enumerate(H2):
                nc.sync.dma_start(W2g_t[i][:], W2g_d[k0:k1, :])

            def bias_tile(d, nm):
                t = const.tile([d.shape[0], d.shape[1]], F32, name=nm, tag=nm)
                nc.sync.dma_start(t[:], d[:])
                return t
            b1a_t = bias_tile(b1a_d, "b1a")
            b2a_t = bias_tile(b2a_d, "b2a")
            b1c_t = bias_tile(b1c_d, "b1c")
            b2c_t = bias_tile(b2c_d, "b2c")
            b1g_t = bias_tile(b1g_d, "b1g")
            b2g_t = bias_tile(b2g_d, "b2g")

            sT_t = const.tile([P, 4 * NB], I32)
            nc.sync.dma_start(sT_t[:], sT_d[:])
            lmT_t = const.tile([P, 4 * NB], F32)
            nc.sync.dma_start(lmT_t[:], lmT_d[:])
            lenf_t = const.tile([1, 2 * NB], I32)
            nc.sync.dma_start(lenf_t[:], lenf_d[:])
            lmg_t = const.tile([1, 2 * NB], BF16)
            nc.sync.dma_start(lmg_t[:], lmg_d[:])

            # v accumulators [100, NB] per (sentence, m-chunk)
            v_all = [[const.tile([100, NB], F32, name=f"v{s}{m}", tag=f"v{s}{m}")
                      for m in range(2)] for s in range(2)]

            # ---- c0 (compare output at an all-zero input column) + corr ----
            rb1c = const.tile([100, 2], BF16)
            nc.scalar.activation(rb1c[:], b1c_t[:], ACTF.Relu, bias=0.0, scale=1.0)
            c0T = []
            for m, (m0, m1) in enumerate(H2):
                cps = psMLP.tile([100, 1], F32, name="c0_ps", tag="mlp")
                for k in range(2):
                    nc.tensor.matmul(cps[:], W2c_t[k][:, m0:m1], rb1c[:, k:k + 1],
                                     start=(k == 0), stop=(k == 1))
                c0m = const.tile([100, 1], F32, name=f"c0{m}", tag=f"c0{m}")
                nc.scalar.activation(c0m[:], cps[:], ACTF.Relu, bias=b2c_t[:, m:m + 1], scale=1.0)
                # transpose to [1, 100] for the rank-1 correction matmul
                tps = psSC.tile([1, P], F32, name="c0T_ps", tag="sc")
                nc.tensor.transpose(tps[:, :100], c0m[:], ident_f[:100, :100])
                c0t = const.tile([1, 100], BF16, name=f"c0T{m}", tag=f"c0T{m}")
                nc.vector.tensor_copy(c0t[:], tps[:, :100])
                c0T.append(c0t)
            corr = [[None, None], [None, None]]
            for s in range(2):
                for m in range(2):
                    cps = psMLP.tile([100, NB], F32, name="corr_ps", tag="mlp")
                    nc.tensor.matmul(cps[:], c0T[m][:], lmg_t[:, s * NB:(s + 1) * NB],
                                     start=True, stop=True)
                    ct = const.tile([100, NB], F32, name=f"corr{s}{m}", tag=f"corr{s}{m}")
                    nc.vector.tensor_copy(ct[:], cps[:])
                    corr[s][m] = ct

            def scol(s, c):  # column base in sT/lmT for (sentence, L-chunk)
                return (s * 2 + c) * NB

            # ---------------- group loop ----------------
            for g in range(NG):
                b0 = g * GB
                # --- gathers: eR[b4][s][c] = emb rows for tokens ---
                eR = [[[None, None] for _ in range(2)] for _ in range(GB)]
                for b4 in range(GB):
                    for s in range(2):
                        for c in range(2):
                            t = gat.tile([P, EMBED], BF16, name=f"eR{b4}{s}{c}", tag=f"eR{b4}{s}{c}")
                            nc.gpsimd.indirect_dma_start(
                                out=t[:], out_offset=None, in_=emb_d[:],
                                in_offset=bass.IndirectOffsetOnAxis(
                                    ap=sT_t[:, scol(s, c) + b0 + b4: scol(s, c) + b0 + b4 + 1],
                                    axis=0),
                            )
                            eR[b4][s][c] = t

                # --- transposes -> eT_sb[s][k] [kw, 1024] bf16 ---
                eT_sb = [[None] * 3 for _ in range(2)]
                for s in range(2):
                    for k, (k0, k1) in enumerate(EK):
                        kw = k1 - k0
                        et = eTp.tile([kw, GB * L], BF16, name=f"eT{s}{k}", tag=f"eT{s}{k}")
                        for h in range(2):
                            tp = psSC.tile([P, 512], BF16, name="tr_ps", tag="sc")
                            for q in range(4):
                                b4, c = divmod(h * 4 + q, 2)
                                nc.tensor.transpose(tp[:kw, q * P:(q + 1) * P],
                                                    eR[b4][s][c][:, k0:k1], ident[:])
                            nc.any.tensor_copy(et[:, h * 512:(h + 1) * 512], tp[:kw, :])
                        eT_sb[s][k] = et

                # --- attend L1: ha[s][m] [100, 1024] bf16 ---
                ha = [[None, None] for _ in range(2)]
                hT = [[None, None] for _ in range(2)]
                for m, (m0, m1) in enumerate(H2):
                    for h in range(2):
                        pp = [psMLP.tile([100, 512], F32, name=f"a1_ps{s}", tag="mlp")
                              for s in range(2)]
                        for k in range(3):
                            for s in range(2):
                                nc.tensor.matmul(pp[s][:], W1a_t[k][:, m0:m1],
                                                 eT_sb[s][k][:, h * 512:(h + 1) * 512],
                                                 start=(k == 0), stop=(k == 2))
                        for s in range(2):
                            if ha[s][m] is None:
                                ha[s][m] = hp.tile([100, GB * L], BF16, name=f"ha{s}{m}", tag=f"ha{s}{m}")
                            nc.scalar.activation(ha[s][m][:, h * 512:(h + 1) * 512], pp[s][:],
                                                 ACTF.Relu, bias=b1a_t[:, m:m + 1], scale=1.0)
                # --- attend L2: hT[s][m] [100, 1024] bf16 ---
                for m, (m0, m1) in enumerate(H2):
                    for h in range(2):
                        qp = [psMLP.tile([100, 512], F32, name=f"a2_ps{s}", tag="mlp")
                              for s in range(2)]
                        for k2 in range(2):
                            for s in range(2):
                                nc.tensor.matmul(qp[s][:], W2a_t[k2][:, m0:m1],
                                                 ha[s][k2][:, h * 512:(h + 1) * 512],
                                                 start=(k2 == 0), stop=(k2 == 1))
                        for s in range(2):
                            if hT[s][m] is None:
                                hT[s][m] = hp.tile([100, GB * L], BF16, name=f"hT{s}{m}", tag=f"hT{s}{m}")
                            nc.scalar.activation(hT[s][m][:, h * 512:(h + 1) * 512], qp[s][:],
                                                 ACTF.Relu, bias=b2a_t[:, m:m + 1], scale=1.0)

                # --- per-batch: scores -> G -> exp -> den -> attn -> xT ---
                # software-pipelined by one batch so the PE never waits on exp.
                xsb = [[None] * 3 for _ in range(2)]   # compare x-inputs per sentence
                for s in range(2):
                    for k, (k0, k1) in enumerate(EK):
                        xsb[s][k] = eTp.tile([k1 - k0, GB * L], BF16, name=f"x{s}{k}", tag=f"x{s}{k}")

                pe_t, pet_t, u_t, bias_t = {}, {}, {}, {}

                def emit_scores(b4):
                    bcol = b4 * L
                    pe = psSC.tile([P, 512], F32, name="pe", tag="sc")
                    pet = psSC.tile([P, 512], F32, name="pet", tag="sc")
                    for ic in range(2):
                        for m in range(2):
                            nc.tensor.matmul(pe[:, ic * L:(ic + 1) * L],
                                             hT[0][m][:, bcol + ic * P: bcol + (ic + 1) * P],
                                             hT[1][m][:, bcol:bcol + L],
                                             start=(m == 0), stop=(m == 1))
                    for jc in range(2):
                        for m in range(2):
                            nc.tensor.matmul(pet[:, jc * L:(jc + 1) * L],
                                             hT[1][m][:, bcol + jc * P: bcol + (jc + 1) * P],
                                             hT[0][m][:, bcol:bcol + L],
                                             start=(m == 0), stop=(m == 1))
                    pe_t[b4], pet_t[b4] = pe, pet

                def emit_softmax(b4):
                    b = b0 + b4
                    pe, pet = pe_t[b4], pet_t[b4]
                    # global max G over e (covers eT too)
                    mx = sm.tile([P, 1], F32, name="mx", tag="mx")
                    nc.vector.tensor_reduce(mx[:], pe[:], axis=AX.X, op=ALU.max)
                    Gb = sm.tile([P, 1], F32, name="Gb", tag="Gb")
                    nc.gpsimd.partition_all_reduce(Gb[:], mx[:], channels=P,
                                                   reduce_op=bass_isa.ReduceOp.max)
                    # exp biases: lmT - G  (per direction d, chunk c)
                    bias_t[b4] = {}
                    for d in range(2):
                        for c in range(2):
                            bt = sm.tile([P, 1], F32, name=f"bx{d}{c}", tag=f"bx{d}{c}")
                            nc.vector.tensor_tensor(
                                bt[:], lmT_t[:, scol(d, c) + b: scol(d, c) + b + 1],
                                Gb[:], op=ALU.subtract)
                            bias_t[b4][(d, c)] = bt
                    # exp from PSUM -> u bf16
                    u_t[b4] = {}
                    for d, src in ((0, pe), (1, pet)):
                        for c in range(2):
                            ut = up.tile([P, L], BF16, name=f"u{b4}{d}{c}", tag=f"u{b4}{d}{c}")
                            nc.scalar.activation(ut[:], src[:, c * L:(c + 1) * L], ACTF.Exp,
                                                 bias=bias_t[b4][(d, c)][:], scale=1.0)
                            u_t[b4][(d, c)] = ut

                def emit_attn(b4):
                    b = b0 + b4
                    bcol = b4 * L
                    u = u_t[b4]
                    # denominators (both directions in one PSUM tile)
                    den = psSC.tile([1, 512], F32, name="den", tag="sc")
                    for d in range(2):
                        for c in range(2):
                            nc.tensor.matmul(den[:, d * L:(d + 1) * L], ones_col[:], u[(d, c)][:],
                                             start=(c == 0), stop=(c == 1))
                    denE = sm.tile([1, 512], F32, name="denE", tag="denE")
                    nc.vector.tensor_scalar_add(denE[:], den[:], 1e-30)
                    rc = sm.tile([1, 512], F32, name="rc", tag="rc")
                    nc.vector.reciprocal(rc[:], denE[:])
                    R = []
                    for d in range(2):
                        # mask for the *output* positions of direction d
                        mr = sm.tile([1, L], F32, name=f"mr{d}", tag=f"mr{d}")
                        nc.vector.tensor_tensor(
                            mr[:], iota_row[:],
                            lenf_t[:, (1 - d) * NB + b:(1 - d) * NB + b + 1].to_broadcast([1, L]),
                            op=ALU.is_lt)
                        rm = sm.tile([1, L], F32, name=f"rm{d}", tag=f"rm{d}")
                        nc.vector.tensor_tensor(rm[:], rc[:, d * L:(d + 1) * L], mr[:], op=ALU.mult)
                        Rb = sm.tile([P, L], F32, name=f"R{d}", tag=f"R{d}")
                        nc.gpsimd.partition_broadcast(Rb[:], rm[:])
                        R.append(Rb)
                    # attention sums + normalization into xsb (x for sentence 1-d)
                    for d in range(2):
                        for k, (k0, k1) in enumerate(EK):
                            kw = k1 - k0
                            ap_ = psMLP.tile([P, 512], F32, name="attn_ps", tag="mlp")
                            for c in range(2):
                                nc.tensor.matmul(ap_[:kw, :L], eR[b4][d][c][:, k0:k1], u[(d, c)][:],
                                                 start=(c == 0), stop=(c == 1))
                            nc.vector.tensor_tensor(xsb[1 - d][k][:, bcol:bcol + L],
                                                    ap_[:kw, :L], R[d][:kw, :], op=ALU.mult)

                emit_scores(0)
                emit_softmax(0)
                for b4 in range(1, GB):
                    emit_scores(b4)
                    emit_attn(b4 - 1)
                    emit_softmax(b4)
                emit_attn(GB - 1)

                # --- compare L1: r1[s][m] [100, 1024] bf16 ---
                r1 = [[None, None] for _ in range(2)]
                for m, (m0, m1) in enumerate(H2):
                    for h in range(2):
                        cp = [psMLP.tile([100, 512], F32, name=f"c1_ps{s}", tag="mlp")
                              for s in range(2)]
                        for k in range(3):
                            for s in range(2):
                                nc.tensor.matmul(cp[s][:], W1ca_t[k][:, m0:m1],
                                                 eT_sb[s][k][:, h * 512:(h + 1) * 512],
                                                 start=(k == 0), stop=False)
                        for k in range(3):
                            for s in range(2):
                                nc.tensor.matmul(cp[s][:], W1cb_t[k][:, m0:m1],
                                                 xsb[s][k][:, h * 512:(h + 1) * 512],
                                                 start=False, stop=(k == 2))
                        for s in range(2):
                            if r1[s][m] is None:
                                r1[s][m] = cmp_.tile([100, GB * L], BF16, name=f"r1{s}{m}", tag=f"r1{s}{m}")
                            reg = r1[s][m][:, h * 512:(h + 1) * 512]
                            nc.vector.tensor_tensor(reg, cp[s][:],
                                                    b1c_t[:, m:m + 1].to_broadcast([100, 512]),
                                                    op=ALU.add)
                            nc.vector.tensor_scalar_max(reg, reg, 0.0)
                # --- compare L2 + masked-sum into v (accum_out) ---
                for m, (m0, m1) in enumerate(H2):
                    for h in range(2):
                        cq = [psMLP.tile([100, 512], F32, name=f"c2_ps{s}", tag="mlp")
                              for s in range(2)]
                        for k2 in range(2):
                            for s in range(2):
                                nc.tensor.matmul(cq[s][:], W2c_t[k2][:, m0:m1],
                                                 r1[s][k2][:, h * 512:(h + 1) * 512],
                                                 start=(k2 == 0), stop=(k2 == 1))
                        for s in range(2):
                            for q in range(2):
                                b4 = h * 2 + q
                                scr = cmp_.tile([100, L], BF16, name="c2scr", tag=f"c2scr{s}")
                                nc.scalar.activation(
                                    scr[:], cq[s][:, q * L:(q + 1) * L], ACTF.Relu,
                                    bias=b2c_t[:, m:m + 1], scale=1.0,
                                    accum_out=v_all[s][m][:, b0 + b4:b0 + b4 + 1])

            # ---------------- aggregate ----------------
            vb = []
            for s in range(2):
                for m in range(2):
                    t = const.tile([100, NB], BF16, name=f"vb{s}{m}", tag=f"vb{s}{m}")
                    nc.vector.tensor_tensor(t[:], v_all[s][m][:], corr[s][m][:], op=ALU.subtract)
                    vb.append(t)
            g1 = []
            for m, (m0, m1) in enumerate(H2):
                gp = psMLP.tile([100, NB], F32, name="g_ps", tag="mlp")
                for k in range(4):
                    nc.tensor.matmul(gp[:], W1g_t[k][:, m0:m1], vb[k][:],
                                     start=(k == 0), stop=(k == 3))
                gt = const.tile([100, NB], BF16, name=f"g1{m}", tag=f"g1{m}")
                nc.scalar.activation(gt[:], gp[:], ACTF.Relu, bias=b1g_t[:, m:m + 1], scale=1.0)
                g1.append(gt)
            op = psMLP.tile([2, NB], F32, name="o_ps", tag="mlp")
            for k2 in range(2):
                nc.tensor.matmul(op[:], W2g_t[k2][:], g1[k2][:],
                                 start=(k2 == 0), stop=(k2 == 1))
            osb = const.tile([2, NB], F32, name="osb", tag="osb")
            nc.scalar.activation(osb[:], op[:], ACTF.Identity, bias=b2g_t[:], scale=1.0)
            nc.sync.dma_start(out_d[:].rearrange("b o -> o b"), osb[:])

    nc.compile()
    return nc


def _shard_inputs(inputs, ncores=NCORES):
    import ml_dtypes
    BF = ml_dtypes.bfloat16
    f = np.ascontiguousarray

    emb = np.zeros((VOCAB + 1, EMBED), dtype=BF)
    emb[:VOCAB] = inputs['emb'].astype(BF)

    Wb = {k: f(inputs[k].astype(BF)) for k in ('W1a', 'W2a', 'W1c', 'W2c', 'W1g', 'W2g')}
    bias = {k: f(inputs[k].astype(np.float32).reshape(2, 100).T)
            for k in ('b1a', 'b2a', 'b1c', 'b2c', 'b1g')}
    b2g = f(inputs['b2g'].astype(np.float32).reshape(2, 1))

    pos = np.arange(L)
    maps = []
    for cid in range(ncores):
        sl = slice(cid * NB, (cid + 1) * NB)
        s = [inputs['s1'][sl].astype(np.int32), inputs['s2'][sl].astype(np.int32)]
        ln = [inputs['len1'][sl].astype(np.int32), inputs['len2'][sl].astype(np.int32)]
        sT = np.zeros((128, 4 * NB), dtype=np.int32)
        lmT = np.zeros((128, 4 * NB), dtype=np.float32)
        lenf = np.zeros((1, 2 * NB), dtype=np.int32)
        lmg = np.zeros((1, 2 * NB), dtype=BF)
        for si in range(2):
            valid = pos[None, :] < ln[si][:, None]          # [NB, L]
            sm_ = np.where(valid, s[si], VZERO)
            for c in range(2):
                col = (si * 2 + c) * NB
                sT[:, col:col + NB] = sm_[:, c * 128:(c + 1) * 128].T
                lmT[:, col:col + NB] = np.where(valid[:, c * 128:(c + 1) * 128], 0.0, -30000.0).T
            lenf[0, si * NB:(si + 1) * NB] = ln[si]
            lmg[0, si * NB:(si + 1) * NB] = (L - ln[si]).astype(BF)
        maps.append(dict(
            emb=emb, sT=f(sT), lmT=f(lmT), lenf=f(lenf), lmg=f(lmg),
            W1a=Wb['W1a'], W2a=Wb['W2a'], W1c=Wb['W1c'], W2c=Wb['W2c'],
            W1g=Wb['W1g'], W2g=Wb['W2g'],
            b1a=bias['b1a'], b2a=bias['b2a'], b1c=bias['b1c'], b2c=bias['b2c'],
            b1g=bias['b1g'], b2g=b2g,
        ))
    return maps


def kernel(**inputs):
    from concourse.bass_utils import run_bass_kernel_spmd
    if 'prog' not in _prog_cache:
        _prog_cache['prog'] = build_program()
    nc = _prog_cache['prog']
    in_maps = _shard_inputs(inputs)
    res = run_bass_kernel_spmd(nc, in_maps, core_ids=list(range(NCORES)))
    out = np.concatenate([res.results[c]["out"] for c in range(NCORES)], axis=0)
    return out.astype(np.float32)


# revision 11
# speedup vs baseline: 3.0921x; 1.4297x over previous
"""Trainium2 Bass kernel for DecomposableAttention (B=512, L=256, V=50000, E=300, H=200).

Data-parallel over batch across 8 cores (64 batches/core).  All matmuls run in
bf16 (1 cycle/row on the PE vs 4 for fp32) with fp32 PSUM accumulation.
Batches are processed in groups of 4 so the attend/compare MLPs run at
N=512 free dim with the stationary weights reused across consecutive matmuls.

Softmax uses a per-batch *global* max offset (exact softmax invariance per
the reference's own offset freedom), so the max never needs a per-column
broadcast: exp reads score PSUM directly with the length mask and -G folded
into the per-partition ACT bias.  Masked positions gather a zero embedding
row (host-side index masking onto an appended zero row), which zeroes the
compare-MLP inputs at masked columns; the resulting constant output column
is subtracted at the end via a (256-len)*c0 rank-1 correction.
"""
import sys

if '/opt/trn_rl_repo' not in sys.path:
    sys.path.insert(0, '/opt/trn_rl_repo')

import numpy as np

B, L, VOCAB, EMBED, HIDDEN = 512, 256, 50000, 300, 200
NCORES = 8
NB = B // NCORES          # batches per core
GB = 4                    # batches per group
NG = NB // GB             # groups
VZERO = VOCAB             # index of the appended all-zero embedding row

_prog_cache = {}


def build_program():
    import concourse.bass as bass
    import concourse.bass_isa as bass_isa
    import concourse.bacc as bacc
    import concourse.tile as tile
    import concourse.mybir as mybir
    from concourse.masks import make_identity

    F32 = mybir.dt.float32
    BF16 = mybir.dt.bfloat16
    I32 = mybir.dt.int32
    AX = mybir.AxisListType
    ALU = mybir.AluOpType
    ACTF = mybir.ActivationFunctionType
    P = 128
    EK = [(0, 128), (128, 256), (256, 300)]     # E contraction chunks
    H2 = [(0, 100), (100, 200)]                 # H chunks of 100

    nc = bacc.Bacc("TRN2", num_devices=NCORES)

    emb_d = nc.dram_tensor("emb", [VOCAB + 1, EMBED], BF16, kind="ExternalInput")
    sT_d = nc.dram_tensor("sT", [P, 4 * NB], I32, kind="ExternalInput")
    lmT_d = nc.dram_tensor("lmT", [P, 4 * NB], F32, kind="ExternalInput")
    lenf_d = nc.dram_tensor("lenf", [1, 2 * NB], I32, kind="ExternalInput")
    lmg_d = nc.dram_tensor("lmg", [1, 2 * NB], BF16, kind="ExternalInput")
    W1a_d = nc.dram_tensor("W1a", [EMBED, HIDDEN], BF16, kind="ExternalInput")
    W2a_d = nc.dram_tensor("W2a", [HIDDEN, HIDDEN], BF16, kind="ExternalInput")
    W1c_d = nc.dram_tensor("W1c", [2 * EMBED, HIDDEN], BF16, kind="ExternalInput")
    W2c_d = nc.dram_tensor("W2c", [HIDDEN, HIDDEN], BF16, kind="ExternalInput")
    W1g_d = nc.dram_tensor("W1g", [2 * HIDDEN, HIDDEN], BF16, kind="ExternalInput")
    W2g_d = nc.dram_tensor("W2g", [HIDDEN, 2], BF16, kind="ExternalInput")
    b1a_d = nc.dram_tensor("b1a", [100, 2], F32, kind="ExternalInput")
    b2a_d = nc.dram_tensor("b2a", [100, 2], F32, kind="ExternalInput")
    b1c_d = nc.dram_tensor("b1c", [100, 2], F32, kind="ExternalInput")
    b2c_d = nc.dram_tensor("b2c", [100, 2], F32, kind="ExternalInput")
    b1g_d = nc.dram_tensor("b1g", [100, 2], F32, kind="ExternalInput")
    b2g_d = nc.dram_tensor("b2g", [2, 1], F32, kind="ExternalInput")
    out_d = nc.dram_tensor("out", [NB, 2], F32, kind="ExternalOutput")

    with tile.TileContext(nc) as tc:
        import contextlib
        ctx = contextlib.ExitStack()
        with ctx:
            const = ctx.enter_context(tc.tile_pool(name="const", bufs=1))
            gat = ctx.enter_context(tc.tile_pool(name="gat", bufs=2))
            eTp = ctx.enter_context(tc.tile_pool(name="eTp", bufs=2))
            hp = ctx.enter_context(tc.tile_pool(name="hp", bufs=2))
            up = ctx.enter_context(tc.tile_pool(name="up", bufs=2))
            sm = ctx.enter_context(tc.tile_pool(name="sm", bufs=2))
            cmp_ = ctx.enter_context(tc.tile_pool(name="cmp", bufs=2))
            # PSUM: two rings, 4 banks each (pools allocate bufs per TAG, so
            # every tile in a pool shares one tag).
            psMLP = ctx.enter_context(tc.tile_pool(name="psMLP", bufs=4, space="PSUM"))
            psSC = ctx.enter_context(tc.tile_pool(name="psSC", bufs=4, space="PSUM"))

            # ---------------- constants ----------------
            ident_f = const.tile([P, P], F32)
            make_identity(nc, ident_f[:])
            ident = const.tile([P, P], BF16)
            nc.vector.tensor_copy(ident[:], ident_f[:])

            ones_col = const.tile([P, 1], BF16)
            nc.vector.memset(ones_col[:], 1.0)

            iota_row = const.tile([1, L], I32)
            nc.gpsimd.iota(iota_row[:], pattern=[[1, L]], base=0, channel_multiplier=0)

            # weights (stationary tiles, bf16)
            W1a_t = [const.tile([k1 - k0, HIDDEN], BF16, name=f"W1a{i}", tag=f"W1a{i}")
                     for i, (k0, k1) in enumerate(EK)]
            for i, (k0, k1) in enumerate(EK):
                nc.sync.dma_start(W1a_t[i][:], W1a_d[k0:k1, :])
            W2a_t = [const.tile([100, HIDDEN], BF16, name=f"W2a{i}", tag=f"W2a{i}") for i in range(2)]
            for i, (k0, k1) in enumerate(H2):
                nc.sync.dma_start(W2a_t[i][:], W2a_d[k0:k1, :])
            W1ca_t = [const.tile([k1 - k0, HIDDEN], BF16, name=f"W1ca{i}", tag=f"W1ca{i}")
                      for i, (k0, k1) in enumerate(EK)]
            for i, (k0, k1) in enumerate(EK):
                nc.sync.dma_start(W1ca_t[i][:], W1c_d[k0:k1, :])
            W1cb_t = [const.tile([k1 - k0, HIDDEN], BF16, name=f"W1cb{i}", tag=f"W1cb{i}")
                      for i, (k0, k1) in enumerate(EK)]
            for i, (k0, k1) in enumerate(EK):
                nc.sync.dma_start(W1cb_t[i][:], W1c_d[EMBED + k0:EMBED + k1, :])
            W2c_t = [const.tile([100, HIDDEN], BF16, name=f"W2c{i}", tag=f"W2c{i}") for i in range(2)]
            for i, (k0, k1) in enumerate(H2):
                nc.sync.dma_start(W2c_t[i][:], W2c_d[k0:k1, :])
            W1g_t = [const.tile([100, HIDDEN], BF16, name=f"W1g{i}", tag=f"W1g{i}") for i in range(4)]
            for i in range(4):
                nc.sync.dma_start(W1g_t[i][:], W1g_d[i * 100:(i + 1) * 100, :])
            W2g_t = [const.tile([100, 2], BF16, name=f"W2g{i}", tag=f"W2g{i}") for i in range(2)]
            for i, (k0, k1) in enumerate(H2):
                nc.sync.dma_start(W2g_t[i][:], W2g_d[k0:k1, :])

            def bias_tile(d, nm):
                t = const.tile([d.shape[0], d.shape[1]], F32, name=nm, tag=nm)
                nc.sync.dma_start(t[:], d[:])
                return t
            b1a_t = bias_tile(b1a_d, "b1a")
            b2a_t = bias_tile(b2a_d, "b2a")
            b1c_t = bias_tile(b1c_d, "b1c")
            b2c_t = bias_tile(b2c_d, "b2c")
            b1g_t = bias_tile(b1g_d, "b1g")
            b2g_t = bias_tile(b2g_d, "b2g")

            sT_t = const.tile([P, 4 * NB], I32)
            nc.sync.dma_start(sT_t[:], sT_d[:])
            lmT_t = const.tile([P, 4 * NB], F32)
            nc.sync.dma_start(lmT_t[:], lmT_d[:])
            lenf_t = const.tile([1, 2 * NB], I32)
            nc.sync.dma_start(lenf_t[:], lenf_d[:])
            lmg_t = const.tile([1, 2 * NB], BF16)
            nc.sync.dma_start(lmg_t[:], lmg_d[:])

            # v accumulators [100, NB] per (sentence, m-chunk)
            v_all = [[const.tile([100, NB], F32, name=f"v{s}{m}", tag=f"v{s}{m}")
                      for m in range(2)] for s in range(2)]

            # ---- c0 (compare output at an all-zero input column) + corr ----
            rb1c = const.tile([100, 2], BF16)
            nc.scalar.activation(rb1c[:], b1c_t[:], ACTF.Relu, bias=0.0, scale=1.0)
            c0T = []
            for m, (m0, m1) in enumerate(H2):
                cps = psMLP.tile([100, 1], F32, name="c0_ps", tag="mlp")
                for k in range(2):
                    nc.tensor.matmul(cps[:], W2c_t[k][:, m0:m1], rb1c[:, k:k + 1],
                                     start=(k == 0), stop=(k == 1))
                c0m = const.tile([100, 1], F32, name=f"c0{m}", tag=f"c0{m}")
                nc.scalar.activation(c0m[:], cps[:], ACTF.Relu, bias=b2c_t[:, m:m + 1], scale=1.0)
                # transpose to [1, 100] for the rank-1 correction matmul
                tps = psSC.tile([1, P], F32, name="c0T_ps", tag="sc")
                nc.tensor.transpose(tps[:, :100], c0m[:], ident_f[:100, :100])
                c0t = const.tile([1, 100], BF16, name=f"c0T{m}", tag=f"c0T{m}")
                nc.vector.tensor_copy(c0t[:], tps[:, :100])
                c0T.append(c0t)
            corr = [[None, None], [None, None]]
            for s in range(2):
                for m in range(2):
                    cps = psMLP.tile([100, NB], F32, name="corr_ps", tag="mlp")
                    nc.tensor.matmul(cps[:], c0T[m][:], lmg_t[:, s * NB:(s + 1) * NB],
                                     start=True, stop=True)
                    ct = const.tile([100, NB], F32, name=f"corr{s}{m}", tag=f"corr{s}{m}")
                    nc.vector.tensor_copy(ct[:], cps[:])
                    corr[s][m] = ct

            def scol(s, c):  # column base in sT/lmT for (sentence, L-chunk)
                return (s * 2 + c) * NB

            ones_row = const.tile([1, P], BF16)
            nc.vector.memset(ones_row[:], 1.0)

            def emit_gathers(g):
                b0 = g * GB
                eRg = [[[None, None] for _ in range(2)] for _ in range(GB)]
                for s in range(2):
                    for b4 in range(GB):
                        for c in range(2):
                            t = gat.tile([P, EMBED], BF16, name=f"eR{b4}{s}{c}", tag=f"eR{b4}{s}{c}")
                            nc.gpsimd.indirect_dma_start(
                                out=t[:], out_offset=None, in_=emb_d[:],
                                in_offset=bass.IndirectOffsetOnAxis(
                                    ap=sT_t[:, scol(s, c) + b0 + b4: scol(s, c) + b0 + b4 + 1],
                                    axis=0),
                            )
                            eRg[b4][s][c] = t
                return eRg

            # ---------------- group loop ----------------
            eR = emit_gathers(0)
            for g in range(NG):
                b0 = g * GB

                # --- transposes (interleaved with attend L1 below) ---
                eT_sb = [[None] * 3 for _ in range(2)]
                for s in range(2):
                    for k, (k0, k1) in enumerate(EK):
                        eT_sb[s][k] = eTp.tile([k1 - k0, GB * L], BF16,
                                               name=f"eT{s}{k}", tag=f"eT{s}{k}")

                def emit_tr(s, k, h):
                    k0, k1 = EK[k]
                    kw = k1 - k0
                    tp = psSC.tile([P, 512], BF16, name="tr_ps", tag="sc")
                    for q in range(4):
                        b4, c = divmod(h * 4 + q, 2)
                        nc.tensor.transpose(tp[:kw, q * P:(q + 1) * P],
                                            eR[b4][s][c][:, k0:k1], ident[:])
                    nc.any.tensor_copy(eT_sb[s][k][:, h * 512:(h + 1) * 512], tp[:kw, :])

                # --- attend L1 (m=0 pass interleaves the transposes) ---
                ha = [[None, None] for _ in range(2)]
                hT = [[None, None] for _ in range(2)]
                for m, (m0, m1) in enumerate(H2):
                    for h in range(2):
                        if m == 0:
                            emit_tr(0, 0, h)
                            emit_tr(1, 0, h)
                        pp = [psMLP.tile([100, 512], F32, name=f"a1_ps{s}", tag="mlp")
                              for s in range(2)]
                        for k in range(3):
                            if m == 0 and k + 1 < 3:
                                emit_tr(0, k + 1, h)
                                emit_tr(1, k + 1, h)
                            for s in range(2):
                                nc.tensor.matmul(pp[s][:], W1a_t[k][:, m0:m1],
                                                 eT_sb[s][k][:, h * 512:(h + 1) * 512],
                                                 start=(k == 0), stop=(k == 2))
                        for s in range(2):
                            if ha[s][m] is None:
                                ha[s][m] = hp.tile([100, GB * L], BF16, name=f"ha{s}{m}", tag=f"ha{s}{m}")
                            nc.scalar.activation(ha[s][m][:, h * 512:(h + 1) * 512], pp[s][:],
                                                 ACTF.Relu, bias=b1a_t[:, m:m + 1], scale=1.0)
                # --- attend L2: hT[s][m] [100, 1024] bf16 ---
                for m, (m0, m1) in enumerate(H2):
                    for h in range(2):
                        qp = [psMLP.tile([100, 512], F32, name=f"a2_ps{s}", tag="mlp")
                              for s in range(2)]
                        for k2 in range(2):
                            for s in range(2):
                                nc.tensor.matmul(qp[s][:], W2a_t[k2][:, m0:m1],
                                                 ha[s][k2][:, h * 512:(h + 1) * 512],
                                                 start=(k2 == 0), stop=(k2 == 1))
                        for s in range(2):
                            if hT[s][m] is None:
                                hT[s][m] = hp.tile([100, GB * L], BF16, name=f"hT{s}{m}", tag=f"hT{s}{m}")
                            nc.scalar.activation(hT[s][m][:, h * 512:(h + 1) * 512], qp[s][:],
                                                 ACTF.Relu, bias=b2a_t[:, m:m + 1], scale=1.0)

                # --- per-batch: scores -> G -> exp -> den -> attn -> xT ---
                xsb = [[None] * 3 for _ in range(2)]   # compare x-inputs per sentence
                for s in range(2):
                    for k, (k0, k1) in enumerate(EK):
                        xsb[s][k] = eTp.tile([k1 - k0, GB * L], BF16, name=f"x{s}{k}", tag=f"x{s}{k}")

                # output-position masks for all batches, early on gpsimd
                mr_t = {}
                for b4 in range(GB):
                    for si in range(2):
                        mr = sm.tile([1, L], F32, name=f"mr{b4}{si}", tag=f"mr{b4}{si}")
                        nc.vector.tensor_tensor(
                            mr[:], iota_row[:],
                            lenf_t[:, si * NB + b0 + b4: si * NB + b0 + b4 + 1].to_broadcast([1, L]),
                            op=ALU.is_lt)
                        mr_t[(b4, si)] = mr

                pe_t, pet_t, u_t, bias_t = {}, {}, {}, {}

                def emit_scores(b4):
                    bcol = b4 * L
                    pe = psSC.tile([P, 512], F32, name="pe", tag="sc")
                    pet = psSC.tile([P, 512], F32, name="pet", tag="sc")
                    for ic in range(2):
                        for m in range(2):
                            nc.tensor.matmul(pe[:, ic * L:(ic + 1) * L],
                                             hT[0][m][:, bcol + ic * P: bcol + (ic + 1) * P],
                                             hT[1][m][:, bcol:bcol + L],
                                             start=(m == 0), stop=(m == 1))
                    for jc in range(2):
                        for m in range(2):
                            nc.tensor.matmul(pet[:, jc * L:(jc + 1) * L],
                                             hT[1][m][:, bcol + jc * P: bcol + (jc + 1) * P],
                                             hT[0][m][:, bcol:bcol + L],
                                             start=(m == 0), stop=(m == 1))
                    pe_t[b4], pet_t[b4] = pe, pet

                def emit_softmax(b4):
                    b = b0 + b4
                    pe, pet = pe_t[b4], pet_t[b4]
                    # global max G over e (covers eT too)
                    mx = sm.tile([P, 1], F32, name="mx", tag="mx")
                    nc.vector.tensor_reduce(mx[:], pe[:], axis=AX.X, op=ALU.max)
                    Gb = sm.tile([P, 1], F32, name="Gb", tag="Gb")
                    nc.gpsimd.partition_all_reduce(Gb[:], mx[:], channels=P,
                                                   reduce_op=bass_isa.ReduceOp.max)
                    # exp biases: lmT - G  (per direction d, chunk c)
                    bias_t[b4] = {}
                    for d in range(2):
                        for c in range(2):
                            bt = sm.tile([P, 1], F32, name=f"bx{d}{c}", tag=f"bx{d}{c}")
                            nc.vector.tensor_tensor(
                                bt[:], lmT_t[:, scol(d, c) + b: scol(d, c) + b + 1],
                                Gb[:], op=ALU.subtract)
                            bias_t[b4][(d, c)] = bt
                    # exp from PSUM -> u bf16
                    u_t[b4] = {}
                    for d, src in ((0, pe), (1, pet)):
                        for c in range(2):
                            ut = up.tile([P, L], BF16, name=f"u{b4}{d}{c}", tag=f"u{b4}{d}{c}")
                            nc.scalar.activation(ut[:], src[:, c * L:(c + 1) * L], ACTF.Exp,
                                                 bias=bias_t[b4][(d, c)][:], scale=1.0)
                            u_t[b4][(d, c)] = ut

                def emit_attn(b4):
                    bcol = b4 * L
                    u = u_t[b4]
                    # denominators (both directions in one PSUM tile)
                    den = psSC.tile([1, 512], F32, name="den", tag="sc")
                    for d in range(2):
                        for c in range(2):
                            nc.tensor.matmul(den[:, d * L:(d + 1) * L], ones_col[:], u[(d, c)][:],
                                             start=(c == 0), stop=(c == 1))
                    rc = sm.tile([1, 512], F32, name="rc", tag="rc")
                    nc.vector.reciprocal_approx_fast(rc[:], den[:])
                    # rm = recip * mask, bf16; R = broadcast via PE matmul,
                    # evacuated to SBUF (DVE can read only one PSUM operand)
                    Rp = psMLP.tile([P, 512], F32, name="R_ps", tag="mlp")
                    for d in range(2):
                        rm = sm.tile([1, L], BF16, name=f"rm{d}", tag=f"rm{d}")
                        nc.vector.tensor_tensor(rm[:], rc[:, d * L:(d + 1) * L],
                                                mr_t[(b4, 1 - d)][:], op=ALU.mult)
                        nc.tensor.matmul(Rp[:, d * L:(d + 1) * L], ones_row[:], rm[:],
                                         start=True, stop=True)
                    Rs = sm.tile([P, 512], BF16, name="Rs", tag="Rs")
                    nc.any.tensor_copy(Rs[:], Rp[:])
                    # attention sums + normalization into xsb (x for sentence 1-d)
                    for d in range(2):
                        for k, (k0, k1) in enumerate(EK):
                            kw = k1 - k0
                            ap_ = psMLP.tile([P, 512], F32, name="attn_ps", tag="mlp")
                            for c in range(2):
                                nc.tensor.matmul(ap_[:kw, :L], eR[b4][d][c][:, k0:k1], u[(d, c)][:],
                                                 start=(c == 0), stop=(c == 1))
                            nc.vector.tensor_tensor(xsb[1 - d][k][:, bcol:bcol + L],
                                                    ap_[:kw, :L], Rs[:kw, d * L:(d + 1) * L],
                                                    op=ALU.mult)

                r1 = [[None, None] for _ in range(2)]
                for s in range(2):
                    for m in range(2):
                        r1[s][m] = cmp_.tile([100, GB * L], BF16, name=f"r1{s}{m}", tag=f"r1{s}{m}")

                def emit_compare(h):
                    # compare L1 for half h (batches 2h, 2h+1)
                    for m, (m0, m1) in enumerate(H2):
                        cp = [psMLP.tile([100, 512], F32, name=f"c1_ps{s}", tag="mlp")
                              for s in range(2)]
                        for k in range(3):
                            for s in range(2):
                                nc.tensor.matmul(cp[s][:], W1ca_t[k][:, m0:m1],
                                                 eT_sb[s][k][:, h * 512:(h + 1) * 512],
                                                 start=(k == 0), stop=False)
                        for k in range(3):
                            for s in range(2):
                                nc.tensor.matmul(cp[s][:], W1cb_t[k][:, m0:m1],
                                                 xsb[s][k][:, h * 512:(h + 1) * 512],
                                                 start=False, stop=(k == 2))
                        for s in range(2):
                            reg = r1[s][m][:, h * 512:(h + 1) * 512]
                            nc.vector.tensor_tensor(reg, cp[s][:],
                                                    b1c_t[:, m:m + 1].to_broadcast([100, 512]),
                                                    op=ALU.add)
                            nc.vector.tensor_scalar_max(reg, reg, 0.0)
                    # compare L2 + masked-sum into v (accum_out)
                    for m, (m0, m1) in enumerate(H2):
                        cq = [psMLP.tile([100, 512], F32, name=f"c2_ps{s}", tag="mlp")
                              for s in range(2)]
                        for k2 in range(2):
                            for s in range(2):
                                nc.tensor.matmul(cq[s][:], W2c_t[k2][:, m0:m1],
                                                 r1[s][k2][:, h * 512:(h + 1) * 512],
                                                 start=(k2 == 0), stop=(k2 == 1))
                        for s in range(2):
                            for q in range(2):
                                b4 = h * 2 + q
                                scr = cmp_.tile([100, L], BF16, name="c2scr", tag=f"c2scr{s}")
                                nc.scalar.activation(
                                    scr[:], cq[s][:, q * L:(q + 1) * L], ACTF.Relu,
                                    bias=b2c_t[:, m:m + 1], scale=1.0,
                                    accum_out=v_all[s][m][:, b0 + b4:b0 + b4 + 1])

                emit_scores(0)
                emit_softmax(0)
                emit_scores(1)
                emit_attn(0)
                emit_softmax(1)
                emit_scores(2)
                emit_attn(1)
                emit_softmax(2)
                emit_compare(0)
                emit_scores(3)
                emit_attn(2)
                emit_softmax(3)
                emit_attn(3)
                if g + 1 < NG:
                    eR_next = emit_gathers(g + 1)
                else:
                    eR_next = None
                emit_compare(1)
                eR = eR_next

            # ---------------- aggregate ----------------
            vb = []
            for s in range(2):
                for m in range(2):
                    t = const.tile([100, NB], BF16, name=f"vb{s}{m}", tag=f"vb{s}{m}")
                    nc.vector.tensor_tensor(t[:], v_all[s][m][:], corr[s][m][:], op=ALU.subtract)
                    vb.append(t)
            g1 = []
            for m, (m0, m1) in enumerate(H2):
                gp = psMLP.tile([100, NB], F32, name="g_ps", tag="mlp")
                for k in range(4):
                    nc.tensor.matmul(gp[:], W1g_t[k][:, m0:m1], vb[k][:],
                                     start=(k == 0), stop=(k == 3))
                gt = const.tile([100, NB], BF16, name=f"g1{m}", tag=f"g1{m}")
                nc.scalar.activation(gt[:], gp[:], ACTF.Relu, bias=b1g_t[:, m:m + 1], scale=1.0)
                g1.append(gt)
            op = psMLP.tile([2, NB], F32, name="o_ps", tag="mlp")
            for k2 in range(2):
                nc.tensor.matmul(op[:], W2g_t[k2][:], g1[k2][:],
                                 start=(k2 == 0), stop=(k2 == 1))
            osb = const.tile([2, NB], F32, name="osb", tag="osb")
            nc.scalar.activation(osb[:], op[:], ACTF.Identity, bias=b2g_t[:], scale=1.0)
            nc.sync.dma_start(out_d[:].rearrange("b o -> o b"), osb[:])

    nc.compile()
    return nc


def _shard_inputs(inputs, ncores=NCORES):
    import ml_dtypes
    BF = ml_dtypes.bfloat16
    f = np.ascontiguousarray

    emb = np.zeros((VOCAB + 1, EMBED), dtype=BF)
    emb[:VOCAB] = inputs['emb'].astype(BF)

    Wb = {k: f(inputs[k].astype(BF)) for k in ('W1a', 'W2a', 'W1c', 'W2c', 'W1g', 'W2g')}
    bias = {k: f(inputs[k].astype(np.float32).reshape(2, 100).T)
            for k in ('b1a', 'b2a', 'b1c', 'b2c', 'b1g')}
    b2g = f(inputs['b2g'].astype(np.float32).reshape(2, 1))

    pos = np.arange(L)
    maps = []
    for cid in range(ncores):
        sl = slice(cid * NB, (cid + 1) * NB)
        s = [inputs['s1'][sl].astype(np.int32), inputs['s2'][sl].astype(np.int32)]
        ln = [inputs['len1'][sl].astype(np.int32), inputs['len2'][sl].astype(np.int32)]
        sT = np.zeros((128, 4 * NB), dtype=np.int32)
        lmT = np.zeros((128, 4 * NB), dtype=np.float32)
        lenf = np.zeros((1, 2 * NB), dtype=np.int32)
        lmg = np.zeros((1, 2 * NB), dtype=BF)
        for si in range(2):
            valid = pos[None, :] < ln[si][:, None]          # [NB, L]
            sm_ = np.where(valid, s[si], VZERO)
            for c in range(2):
                col = (si * 2 + c) * NB
                sT[:, col:col + NB] = sm_[:, c * 128:(c + 1) * 128].T
                lmT[:, col:col + NB] = np.where(valid[:, c * 128:(c + 1) * 128], 0.0, -30000.0).T
            lenf[0, si * NB:(si + 1) * NB] = ln[si]
            lmg[0, si * NB:(si + 1) * NB] = (L - ln[si]).astype(BF)
        maps.append(dict(
            emb=emb, sT=f(sT), lmT=f(lmT), lenf=f(lenf), lmg=f(lmg),
            W1a=Wb['W1a'], W2a=Wb['W2a'], W1c=Wb['W1c'], W2c=Wb['W2c'],
            W1g=Wb['W1g'], W2g=Wb['W2g'],
            b1a=bias['b1a'], b2a=bias['b2a'], b1c=bias['b1c'], b2c=bias['b2c'],
            b1g=bias['b1g'], b2g=b2g,
        ))
    return maps


def kernel(**inputs):
    from concourse.bass_utils import run_bass_kernel_spmd
    if 'prog' not in _prog_cache:
        _prog_cache['prog'] = build_program()
    nc = _prog_cache['prog']
    in_maps = _shard_inputs(inputs)
    res = run_bass_kernel_spmd(nc, in_maps, core_ids=list(range(NCORES)))
    out = np.concatenate([res.results[c]["out"] for c in range(NCORES)], axis=0)
    return out.astype(np.float32)


# revision 12
# speedup vs baseline: 3.4858x; 1.1273x over previous
"""Trainium2 Bass kernel for DecomposableAttention (B=512, L=256, V=50000, E=300, H=200).

v4: v3 (bf16 PE, 4-batch groups, global-max softmax, zero-row gather,
rank-1 masked-sum correction) + length-class specialization: batches are
host-sorted by (ceil(len1/128), ceil(len2/128)) into classes (1,1), (1,2),
(2,1), (2,2); each group of 4 batches shares a class and only processes the
live 128-position chunks.  Class counts are rounded to multiples of 32 (8
cores x 4 batches) by upgrading leftovers to a superset class, so all cores
run the same program.  The host un-permutes the output rows.
"""
import sys

if '/opt/trn_rl_repo' not in sys.path:
    sys.path.insert(0, '/opt/trn_rl_repo')

import numpy as np

B, L, VOCAB, EMBED, HIDDEN = 512, 256, 50000, 300, 200
NCORES = 8
NB = B // NCORES          # batches per core
GB = 4                    # batches per group
NG = NB // GB             # groups
VZERO = VOCAB             # index of the appended all-zero embedding row

_prog_cache = {}


def make_schedule(len1, len2):
    """Global batch -> per-core permutation + shared group class schedule."""
    ci = np.minimum((len1.astype(np.int64) + 127) // 128, 2)
    cj = np.minimum((len2.astype(np.int64) + 127) // 128, 2)
    buckets = {(1, 1): [], (1, 2): [], (2, 1): [], (2, 2): []}
    for idx in range(len(len1)):
        buckets[(int(ci[idx]), int(cj[idx]))].append(idx)
    unit = NCORES * GB
    for src, dst in [((1, 1), (1, 2)), ((2, 1), (2, 2)), ((1, 2), (2, 2))]:
        keep = len(buckets[src]) - (len(buckets[src]) % unit)
        buckets[dst] = buckets[src][keep:] + buckets[dst]
        buckets[src] = buckets[src][:keep]
    assert len(buckets[(2, 2)]) % unit == 0
    percore = [[] for _ in range(NCORES)]
    classes = [[] for _ in range(NCORES)]   # class per batch slot
    sched = []
    for c in [(1, 1), (1, 2), (2, 1), (2, 2)]:
        lst = buckets[c]
        n = len(lst) // NCORES
        for core in range(NCORES):
            percore[core] += lst[core * n:(core + 1) * n]
            classes[core] += [c] * n
        sched += [c] * (n // GB)
    assert len(sched) == NG
    return percore, classes, sched


def build_program(sched):
    import concourse.bass as bass
    import concourse.bass_isa as bass_isa
    import concourse.bacc as bacc
    import concourse.tile as tile
    import concourse.mybir as mybir
    from concourse.masks import make_identity

    F32 = mybir.dt.float32
    BF16 = mybir.dt.bfloat16
    I32 = mybir.dt.int32
    AX = mybir.AxisListType
    ALU = mybir.AluOpType
    ACTF = mybir.ActivationFunctionType
    P = 128
    EK = [(0, 128), (128, 256), (256, 300)]     # E contraction chunks
    H2 = [(0, 100), (100, 200)]                 # H chunks of 100

    nc = bacc.Bacc("TRN2", num_devices=NCORES)

    emb_d = nc.dram_tensor("emb", [VOCAB + 1, EMBED], BF16, kind="ExternalInput")
    sT_d = nc.dram_tensor("sT", [P, 4 * NB], I32, kind="ExternalInput")
    lmT_d = nc.dram_tensor("lmT", [P, 4 * NB], F32, kind="ExternalInput")
    lenf_d = nc.dram_tensor("lenf", [1, 2 * NB], I32, kind="ExternalInput")
    lmg_d = nc.dram_tensor("lmg", [1, 2 * NB], BF16, kind="ExternalInput")
    W1a_d = nc.dram_tensor("W1a", [EMBED, HIDDEN], BF16, kind="ExternalInput")
    W2a_d = nc.dram_tensor("W2a", [HIDDEN, HIDDEN], BF16, kind="ExternalInput")
    W1c_d = nc.dram_tensor("W1c", [2 * EMBED, HIDDEN], BF16, kind="ExternalInput")
    W2c_d = nc.dram_tensor("W2c", [HIDDEN, HIDDEN], BF16, kind="ExternalInput")
    W1g_d = nc.dram_tensor("W1g", [2 * HIDDEN, HIDDEN], BF16, kind="ExternalInput")
    W2g_d = nc.dram_tensor("W2g", [HIDDEN, 2], BF16, kind="ExternalInput")
    b1a_d = nc.dram_tensor("b1a", [100, 2], F32, kind="ExternalInput")
    b2a_d = nc.dram_tensor("b2a", [100, 2], F32, kind="ExternalInput")
    b1c_d = nc.dram_tensor("b1c", [100, 2], F32, kind="ExternalInput")
    b2c_d = nc.dram_tensor("b2c", [100, 2], F32, kind="ExternalInput")
    b1g_d = nc.dram_tensor("b1g", [100, 2], F32, kind="ExternalInput")
    b2g_d = nc.dram_tensor("b2g", [2, 1], F32, kind="ExternalInput")
    out_d = nc.dram_tensor("out", [NB, 2], F32, kind="ExternalOutput")

    with tile.TileContext(nc) as tc:
        import contextlib
        ctx = contextlib.ExitStack()
        with ctx:
            const = ctx.enter_context(tc.tile_pool(name="const", bufs=1))
            gat = ctx.enter_context(tc.tile_pool(name="gat", bufs=2))
            eTp = ctx.enter_context(tc.tile_pool(name="eTp", bufs=2))
            hp = ctx.enter_context(tc.tile_pool(name="hp", bufs=2))
            up = ctx.enter_context(tc.tile_pool(name="up", bufs=2))
            sm = ctx.enter_context(tc.tile_pool(name="sm", bufs=2))
            cmp_ = ctx.enter_context(tc.tile_pool(name="cmp", bufs=2))
            psMLP = ctx.enter_context(tc.tile_pool(name="psMLP", bufs=4, space="PSUM"))
            psSC = ctx.enter_context(tc.tile_pool(name="psSC", bufs=4, space="PSUM"))

            # ---------------- constants ----------------
            ident_f = const.tile([P, P], F32)
            make_identity(nc, ident_f[:])
            ident = const.tile([P, P], BF16)
            nc.vector.tensor_copy(ident[:], ident_f[:])

            ones_col = const.tile([P, 1], BF16)
            nc.vector.memset(ones_col[:], 1.0)
            ones_row = const.tile([1, P], BF16)
            nc.vector.memset(ones_row[:], 1.0)

            iota_row = const.tile([1, L], I32)
            nc.gpsimd.iota(iota_row[:], pattern=[[1, L]], base=0, channel_multiplier=0)

            # weights (stationary tiles, bf16)
            W1a_t = [const.tile([k1 - k0, HIDDEN], BF16, name=f"W1a{i}", tag=f"W1a{i}")
                     for i, (k0, k1) in enumerate(EK)]
            for i, (k0, k1) in enumerate(EK):
                nc.sync.dma_start(W1a_t[i][:], W1a_d[k0:k1, :])
            W2a_t = [const.tile([100, HIDDEN], BF16, name=f"W2a{i}", tag=f"W2a{i}") for i in range(2)]
            for i, (k0, k1) in enumerate(H2):
                nc.sync.dma_start(W2a_t[i][:], W2a_d[k0:k1, :])
            W1ca_t = [const.tile([k1 - k0, HIDDEN], BF16, name=f"W1ca{i}", tag=f"W1ca{i}")
                      for i, (k0, k1) in enumerate(EK)]
            for i, (k0, k1) in enumerate(EK):
                nc.sync.dma_start(W1ca_t[i][:], W1c_d[k0:k1, :])
            W1cb_t = [const.tile([k1 - k0, HIDDEN], BF16, name=f"W1cb{i}", tag=f"W1cb{i}")
                      for i, (k0, k1) in enumerate(EK)]
            for i, (k0, k1) in enumerate(EK):
                nc.sync.dma_start(W1cb_t[i][:], W1c_d[EMBED + k0:EMBED + k1, :])
            W2c_t = [const.tile([100, HIDDEN], BF16, name=f"W2c{i}", tag=f"W2c{i}") for i in range(2)]
            for i, (k0, k1) in enumerate(H2):
                nc.sync.dma_start(W2c_t[i][:], W2c_d[k0:k1, :])
            W1g_t = [const.tile([100, HIDDEN], BF16, name=f"W1g{i}", tag=f"W1g{i}") for i in range(4)]
            for i in range(4):
                nc.sync.dma_start(W1g_t[i][:], W1g_d[i * 100:(i + 1) * 100, :])
            W2g_t = [const.tile([100, 2], BF16, name=f"W2g{i}", tag=f"W2g{i}") for i in range(2)]
            for i, (k0, k1) in enumerate(H2):
                nc.sync.dma_start(W2g_t[i][:], W2g_d[k0:k1, :])

            def bias_tile(d, nm):
                t = const.tile([d.shape[0], d.shape[1]], F32, name=nm, tag=nm)
                nc.sync.dma_start(t[:], d[:])
                return t
            b1a_t = bias_tile(b1a_d, "b1a")
            b2a_t = bias_tile(b2a_d, "b2a")
            b1c_t = bias_tile(b1c_d, "b1c")
            b2c_t = bias_tile(b2c_d, "b2c")
            b1g_t = bias_tile(b1g_d, "b1g")
            b2g_t = bias_tile(b2g_d, "b2g")

            sT_t = const.tile([P, 4 * NB], I32)
            nc.sync.dma_start(sT_t[:], sT_d[:])
            lmT_t = const.tile([P, 4 * NB], F32)
            nc.sync.dma_start(lmT_t[:], lmT_d[:])
            lenf_t = const.tile([1, 2 * NB], I32)
            nc.sync.dma_start(lenf_t[:], lenf_d[:])
            lmg_t = const.tile([1, 2 * NB], BF16)
            nc.sync.dma_start(lmg_t[:], lmg_d[:])

            v_all = [[const.tile([100, NB], F32, name=f"v{s}{m}", tag=f"v{s}{m}")
                      for m in range(2)] for s in range(2)]

            # ---- c0 (compare output at an all-zero input column) + corr ----
            rb1c = const.tile([100, 2], BF16)
            nc.scalar.activation(rb1c[:], b1c_t[:], ACTF.Relu, bias=0.0, scale=1.0)
            c0T = []
            for m, (m0, m1) in enumerate(H2):
                cps = psMLP.tile([100, 1], F32, name="c0_ps", tag="mlp")
                for k in range(2):
                    nc.tensor.matmul(cps[:], W2c_t[k][:, m0:m1], rb1c[:, k:k + 1],
                                     start=(k == 0), stop=(k == 1))
                c0m = const.tile([100, 1], F32, name=f"c0{m}", tag=f"c0{m}")
                nc.scalar.activation(c0m[:], cps[:], ACTF.Relu, bias=b2c_t[:, m:m + 1], scale=1.0)
                tps = psSC.tile([1, P], F32, name="c0T_ps", tag="sc")
                nc.tensor.transpose(tps[:, :100], c0m[:], ident_f[:100, :100])
                c0t = const.tile([1, 100], BF16, name=f"c0T{m}", tag=f"c0T{m}")
                nc.vector.tensor_copy(c0t[:], tps[:, :100])
                c0T.append(c0t)
            corr = [[None, None], [None, None]]
            for s in range(2):
                for m in range(2):
                    cps = psMLP.tile([100, NB], F32, name="corr_ps", tag="mlp")
                    nc.tensor.matmul(cps[:], c0T[m][:], lmg_t[:, s * NB:(s + 1) * NB],
                                     start=True, stop=True)
                    ct = const.tile([100, NB], F32, name=f"corr{s}{m}", tag=f"corr{s}{m}")
                    nc.vector.tensor_copy(ct[:], cps[:])
                    corr[s][m] = ct

            def scol(s, c):  # column base in sT/lmT for (sentence, L-chunk)
                return (s * 2 + c) * NB

            def emit_gathers(g):
                b0 = g * GB
                I, J = sched[g]
                CH = (I, J)
                eRg = [[[None, None] for _ in range(2)] for _ in range(GB)]
                for s in range(2):
                    for b4 in range(GB):
                        for c in range(CH[s]):
                            t = gat.tile([P, EMBED], BF16, name=f"eR{b4}{s}{c}", tag=f"eR{b4}{s}{c}")
                            nc.gpsimd.indirect_dma_start(
                                out=t[:], out_offset=None, in_=emb_d[:],
                                in_offset=bass.IndirectOffsetOnAxis(
                                    ap=sT_t[:, scol(s, c) + b0 + b4: scol(s, c) + b0 + b4 + 1],
                                    axis=0),
                            )
                            eRg[b4][s][c] = t
                return eRg

            # ---------------- group loop ----------------
            eR = emit_gathers(0)
            for g in range(NG):
                b0 = g * GB
                I, J = sched[g]
                CH = (I, J)                    # chunks per sentence
                LS = (128 * I, 128 * J)        # live positions per sentence

                eT_sb = [[None] * 3 for _ in range(2)]
                for s in range(2):
                    for k, (k0, k1) in enumerate(EK):
                        eT_sb[s][k] = eTp.tile([k1 - k0, GB * L], BF16,
                                               name=f"eT{s}{k}", tag=f"eT{s}{k}")

                def emit_tr(s, k, h):
                    # one PSUM tile = 4 transposed [128,128] quarters = 512 cols
                    k0, k1 = EK[k]
                    kw = k1 - k0
                    nch = CH[s]
                    tp = psSC.tile([P, 512], BF16, name="tr_ps", tag="sc")
                    for q in range(4):
                        pos = h * 4 + q
                        b4, c = divmod(pos, nch)
                        nc.tensor.transpose(tp[:kw, q * P:(q + 1) * P],
                                            eR[b4][s][c][:, k0:k1], ident[:])
                    nc.any.tensor_copy(eT_sb[s][k][:, h * 512:(h + 1) * 512], tp[:kw, :])

                # --- attend L1 (m=0 pass interleaves the transposes) ---
                ha = [[None, None] for _ in range(2)]
                hT = [[None, None] for _ in range(2)]
                for s in range(2):
                    nh = CH[s]                 # halves of 512 cols for this sentence
                    for m, (m0, m1) in enumerate(H2):
                        for h in range(nh):
                            if m == 0:
                                emit_tr(s, 0, h)
                            pp = psMLP.tile([100, 512], F32, name="a1_ps", tag="mlp")
                            for k in range(3):
                                if m == 0 and k + 1 < 3:
                                    emit_tr(s, k + 1, h)
                                nc.tensor.matmul(pp[:], W1a_t[k][:, m0:m1],
                                                 eT_sb[s][k][:, h * 512:(h + 1) * 512],
                                                 start=(k == 0), stop=(k == 2))
                            if ha[s][m] is None:
                                ha[s][m] = hp.tile([100, GB * L], BF16, name=f"ha{s}{m}", tag=f"ha{s}{m}")
                            nc.scalar.activation(ha[s][m][:, h * 512:(h + 1) * 512], pp[:],
                                                 ACTF.Relu, bias=b1a_t[:, m:m + 1], scale=1.0)
                # --- attend L2 ---
                for s in range(2):
                    nh = CH[s]
                    for m, (m0, m1) in enumerate(H2):
                        for h in range(nh):
                            qp = psMLP.tile([100, 512], F32, name="a2_ps", tag="mlp")
                            for k2 in range(2):
                                nc.tensor.matmul(qp[:], W2a_t[k2][:, m0:m1],
                                                 ha[s][k2][:, h * 512:(h + 1) * 512],
                                                 start=(k2 == 0), stop=(k2 == 1))
                            if hT[s][m] is None:
                                hT[s][m] = hp.tile([100, GB * L], BF16, name=f"hT{s}{m}", tag=f"hT{s}{m}")
                            nc.scalar.activation(hT[s][m][:, h * 512:(h + 1) * 512], qp[:],
                                                 ACTF.Relu, bias=b2a_t[:, m:m + 1], scale=1.0)

                # --- per-batch phase ---
                xsb = [[None] * 3 for _ in range(2)]
                for s in range(2):
                    for k, (k0, k1) in enumerate(EK):
                        xsb[s][k] = eTp.tile([k1 - k0, GB * L], BF16, name=f"x{s}{k}", tag=f"x{s}{k}")

                mr_t = {}
                for b4 in range(GB):
                    for si in range(2):
                        mr = sm.tile([1, L], F32, name=f"mr{b4}{si}", tag=f"mr{b4}{si}")
                        nc.vector.tensor_tensor(
                            mr[:], iota_row[:],
                            lenf_t[:, si * NB + b0 + b4: si * NB + b0 + b4 + 1].to_broadcast([1, L]),
                            op=ALU.is_lt)
                        mr_t[(b4, si)] = mr

                pe_t, pet_t, u_t, bias_t = {}, {}, {}, {}

                def emit_scores(b4):
                    bc = (b4 * LS[0], b4 * LS[1])
                    pe = psSC.tile([P, 512], F32, name="pe", tag="sc")
                    pet = psSC.tile([P, 512], F32, name="pet", tag="sc")
                    for ic in range(I):
                        for m in range(2):
                            nc.tensor.matmul(pe[:, ic * LS[1]:(ic + 1) * LS[1]],
                                             hT[0][m][:, bc[0] + ic * P: bc[0] + (ic + 1) * P],
                                             hT[1][m][:, bc[1]:bc[1] + LS[1]],
                                             start=(m == 0), stop=(m == 1))
                    for jc in range(J):
                        for m in range(2):
                            nc.tensor.matmul(pet[:, jc * LS[0]:(jc + 1) * LS[0]],
                                             hT[1][m][:, bc[1] + jc * P: bc[1] + (jc + 1) * P],
                                             hT[0][m][:, bc[0]:bc[0] + LS[0]],
                                             start=(m == 0), stop=(m == 1))
                    pe_t[b4], pet_t[b4] = pe, pet

                def emit_softmax(b4):
                    b = b0 + b4
                    pe, pet = pe_t[b4], pet_t[b4]
                    mx = sm.tile([P, 1], F32, name="mx", tag="mx")
                    nc.vector.tensor_reduce(mx[:], pe[:, :I * LS[1]], axis=AX.X, op=ALU.max)
                    Gb = sm.tile([P, 1], F32, name="Gb", tag="Gb")
                    nc.gpsimd.partition_all_reduce(Gb[:], mx[:], channels=P,
                                                   reduce_op=bass_isa.ReduceOp.max)
                    bias_t[b4] = {}
                    for d in range(2):
                        for c in range(CH[d]):
                            bt = sm.tile([P, 1], F32, name=f"bx{d}{c}", tag=f"bx{d}{c}")
                            nc.vector.tensor_tensor(
                                bt[:], lmT_t[:, scol(d, c) + b: scol(d, c) + b + 1],
                                Gb[:], op=ALU.subtract)
                            bias_t[b4][(d, c)] = bt
                    u_t[b4] = {}
                    for d, src in ((0, pe), (1, pet)):
                        w = LS[1 - d]
                        for c in range(CH[d]):
                            ut = up.tile([P, L], BF16, name=f"u{b4}{d}{c}", tag=f"u{b4}{d}{c}")
                            nc.scalar.activation(ut[:, :w], src[:, c * w:(c + 1) * w], ACTF.Exp,
                                                 bias=bias_t[b4][(d, c)][:], scale=1.0)
                            u_t[b4][(d, c)] = ut

                def emit_attn(b4):
                    bc = (b4 * LS[0], b4 * LS[1])
                    u = u_t[b4]
                    den = psSC.tile([1, 512], F32, name="den", tag="sc")
                    for d in range(2):
                        w = LS[1 - d]
                        for c in range(CH[d]):
                            nc.tensor.matmul(den[:, d * 256:d * 256 + w], ones_col[:],
                                             u[(d, c)][:, :w],
                                             start=(c == 0), stop=(c == CH[d] - 1))
                    rc = sm.tile([1, 512], F32, name="rc", tag="rc")
                    nc.vector.reciprocal_approx_fast(rc[:, :LS[1]], den[:, :LS[1]])
                    nc.vector.reciprocal_approx_fast(rc[:, 256:256 + LS[0]],
                                                     den[:, 256:256 + LS[0]])
                    Rp = psMLP.tile([P, 512], F32, name="R_ps", tag="mlp")
                    for d in range(2):
                        w = LS[1 - d]
                        rm = sm.tile([1, L], BF16, name=f"rm{d}", tag=f"rm{d}")
                        nc.vector.tensor_tensor(rm[:, :w], rc[:, d * 256:d * 256 + w],
                                                mr_t[(b4, 1 - d)][:, :w], op=ALU.mult)
                        nc.tensor.matmul(Rp[:, d * 256:d * 256 + w], ones_row[:], rm[:, :w],
                                         start=True, stop=True)
                    Rs = sm.tile([P, 512], BF16, name="Rs", tag="Rs")
                    nc.any.tensor_copy(Rs[:], Rp[:])
                    for d in range(2):
                        w = LS[1 - d]
                        for k, (k0, k1) in enumerate(EK):
                            kw = k1 - k0
                            ap_ = psMLP.tile([P, 512], F32, name="attn_ps", tag="mlp")
                            for c in range(CH[d]):
                                nc.tensor.matmul(ap_[:kw, :w], eR[b4][d][c][:, k0:k1],
                                                 u[(d, c)][:, :w],
                                                 start=(c == 0), stop=(c == CH[d] - 1))
                            nc.vector.tensor_tensor(xsb[1 - d][k][:, bc[1 - d]:bc[1 - d] + w],
                                                    ap_[:kw, :w], Rs[:kw, d * 256:d * 256 + w],
                                                    op=ALU.mult)

                r1 = [[None, None] for _ in range(2)]
                for s in range(2):
                    for m in range(2):
                        r1[s][m] = cmp_.tile([100, GB * L], BF16, name=f"r1{s}{m}", tag=f"r1{s}{m}")

                def emit_compare(s, h):
                    # compare L1+L2 for sentence s, half h
                    for m, (m0, m1) in enumerate(H2):
                        cp = psMLP.tile([100, 512], F32, name="c1_ps", tag="mlp")
                        for k in range(3):
                            nc.tensor.matmul(cp[:], W1ca_t[k][:, m0:m1],
                                             eT_sb[s][k][:, h * 512:(h + 1) * 512],
                                             start=(k == 0), stop=False)
                        for k in range(3):
                            nc.tensor.matmul(cp[:], W1cb_t[k][:, m0:m1],
                                             xsb[s][k][:, h * 512:(h + 1) * 512],
                                             start=False, stop=(k == 2))
                        reg = r1[s][m][:, h * 512:(h + 1) * 512]
                        nc.vector.tensor_tensor(reg, cp[:],
                                                b1c_t[:, m:m + 1].to_broadcast([100, 512]),
                                                op=ALU.add)
                        nc.vector.tensor_scalar_max(reg, reg, 0.0)
                    segs = 512 // LS[s]
                    for m, (m0, m1) in enumerate(H2):
                        cq = psMLP.tile([100, 512], F32, name="c2_ps", tag="mlp")
                        for k2 in range(2):
                            nc.tensor.matmul(cq[:], W2c_t[k2][:, m0:m1],
                                             r1[s][k2][:, h * 512:(h + 1) * 512],
                                             start=(k2 == 0), stop=(k2 == 1))
                        for q in range(segs):
                            b4 = h * segs + q
                            scr = cmp_.tile([100, L], BF16, name="c2scr", tag=f"c2scr{s}")
                            nc.scalar.activation(
                                scr[:, :LS[s]], cq[:, q * LS[s]:(q + 1) * LS[s]], ACTF.Relu,
                                bias=b2c_t[:, m:m + 1], scale=1.0,
                                accum_out=v_all[s][m][:, b0 + b4:b0 + b4 + 1])

                emit_scores(0)
                emit_softmax(0)
                emit_scores(1)
                emit_attn(0)
                emit_softmax(1)
                emit_scores(2)
                emit_attn(1)
                emit_softmax(2)
                # halves done after batches 0,1: sentence s half h covers
                # batches [h*512//LS[s], ...); emit compare for halves fully
                # covered by batches 0..1
                for s in range(2):
                    if CH[s] == 2:
                        emit_compare(s, 0)
                emit_scores(3)
                emit_attn(2)
                emit_softmax(3)
                emit_attn(3)
                if g + 1 < NG:
                    eR_next = emit_gathers(g + 1)
                else:
                    eR_next = None
                for s in range(2):
                    if CH[s] == 2:
                        emit_compare(s, 1)
                    else:
                        emit_compare(s, 0)
                eR = eR_next

            # ---------------- aggregate ----------------
            vb = []
            for s in range(2):
                for m in range(2):
                    t = const.tile([100, NB], BF16, name=f"vb{s}{m}", tag=f"vb{s}{m}")
                    nc.vector.tensor_tensor(t[:], v_all[s][m][:], corr[s][m][:], op=ALU.subtract)
                    vb.append(t)
            g1 = []
            for m, (m0, m1) in enumerate(H2):
                gp = psMLP.tile([100, NB], F32, name="g_ps", tag="mlp")
                for k in range(4):
                    nc.tensor.matmul(gp[:], W1g_t[k][:, m0:m1], vb[k][:],
                                     start=(k == 0), stop=(k == 3))
                gt = const.tile([100, NB], BF16, name=f"g1{m}", tag=f"g1{m}")
                nc.scalar.activation(gt[:], gp[:], ACTF.Relu, bias=b1g_t[:, m:m + 1], scale=1.0)
                g1.append(gt)
            op = psMLP.tile([2, NB], F32, name="o_ps", tag="mlp")
            for k2 in range(2):
                nc.tensor.matmul(op[:], W2g_t[k2][:], g1[k2][:],
                                 start=(k2 == 0), stop=(k2 == 1))
            osb = const.tile([2, NB], F32, name="osb", tag="osb")
            nc.scalar.activation(osb[:], op[:], ACTF.Identity, bias=b2g_t[:], scale=1.0)
            nc.sync.dma_start(out_d[:].rearrange("b o -> o b"), osb[:])

    nc.compile()
    return nc


def _shard_inputs(inputs, percore, classes):
    import ml_dtypes
    BF = ml_dtypes.bfloat16
    f = np.ascontiguousarray

    emb = np.zeros((VOCAB + 1, EMBED), dtype=BF)
    emb[:VOCAB] = inputs['emb'].astype(BF)

    Wb = {k: f(inputs[k].astype(BF)) for k in ('W1a', 'W2a', 'W1c', 'W2c', 'W1g', 'W2g')}
    bias = {k: f(inputs[k].astype(np.float32).reshape(2, 100).T)
            for k in ('b1a', 'b2a', 'b1c', 'b2c', 'b1g')}
    b2g = f(inputs['b2g'].astype(np.float32).reshape(2, 1))

    pos = np.arange(L)
    maps = []
    for cid in range(NCORES):
        idx = np.array(percore[cid], dtype=np.int64)
        cls = classes[cid]
        s = [inputs['s1'][idx].astype(np.int32), inputs['s2'][idx].astype(np.int32)]
        ln = [inputs['len1'][idx].astype(np.int32), inputs['len2'][idx].astype(np.int32)]
        chunks = np.array([[c[0] for c in cls], [c[1] for c in cls]], dtype=np.int32)  # [2, NB]
        sT = np.zeros((128, 4 * NB), dtype=np.int32)
        lmT = np.zeros((128, 4 * NB), dtype=np.float32)
        lenf = np.zeros((1, 2 * NB), dtype=np.int32)
        lmg = np.zeros((1, 2 * NB), dtype=BF)
        for si in range(2):
            valid = pos[None, :] < ln[si][:, None]          # [NB, L]
            sm_ = np.where(valid, s[si], VZERO)
            for c in range(2):
                col = (si * 2 + c) * NB
                sT[:, col:col + NB] = sm_[:, c * 128:(c + 1) * 128].T
                lmT[:, col:col + NB] = np.where(valid[:, c * 128:(c + 1) * 128], 0.0, -30000.0).T
            lenf[0, si * NB:(si + 1) * NB] = ln[si]
            lmg[0, si * NB:(si + 1) * NB] = (128 * chunks[si] - ln[si]).astype(BF)
        maps.append(dict(
            emb=emb, sT=f(sT), lmT=f(lmT), lenf=f(lenf), lmg=f(lmg),
            W1a=Wb['W1a'], W2a=Wb['W2a'], W1c=Wb['W1c'], W2c=Wb['W2c'],
            W1g=Wb['W1g'], W2g=Wb['W2g'],
            b1a=bias['b1a'], b2a=bias['b2a'], b1c=bias['b1c'], b2c=bias['b2c'],
            b1g=bias['b1g'], b2g=b2g,
        ))
    return maps


def kernel(**inputs):
    from concourse.bass_utils import run_bass_kernel_spmd
    len1 = np.asarray(inputs['len1'])
    len2 = np.asarray(inputs['len2'])
    percore, classes, sched = make_schedule(len1, len2)
    key = tuple(sched)
    if key not in _prog_cache:
        _prog_cache[key] = build_program(sched)
        _prog_cache['last'] = (percore, classes, sched)
    nc = _prog_cache[key]
    in_maps = _shard_inputs(inputs, percore, classes)
    res = run_bass_kernel_spmd(nc, in_maps, core_ids=list(range(NCORES)))
    rows = np.concatenate([res.results[c]["out"] for c in range(NCORES)], axis=0)
    perm = np.concatenate([np.array(p, dtype=np.int64) for p in percore])
    out = np.empty((B, 2), dtype=np.float32)
    out[perm] = rows.astype(np.float32)
    return out
